# revision 1
# baseline (speedup 1.0000x reference)
"""Trainium2 Bass kernel for the DimeNet-style directed-message block.

Reference computation (W = n_angles, E = n_edges, D = 128, A = 49, J = 8):
    m_kj     = m_ji[kj_idx]                          # [W, D]
    transf_m = silu(m_kj @ W_nbr + b_nbr)            # [W, D]
    transf_e = e_rbf[kj_idx] @ W_e                   # [W, D]
    m_and_e  = transf_m * transf_e                   # [W, D]
    transf_a = a_sbf @ W_a                           # [W, J]
    out[w,i] = sum_{j,l} transf_a[w,j] m_and_e[w,l] final_w[i,j,l]
    final    = segment_sum(out, kj_idx, E)           # [E, D]

Algebraic refactor: every per-angle factor except transf_a depends on the
angle only through kj_idx, so the segment sum commutes through the bilinear
form:
    me       = silu(m_ji @ W_nbr + b) * (e_rbf @ W_e)        # [E, D]
    S        = segment_sum(a_sbf @ W_a, kj_idx, E)           # [E, J]
    final[e] = sum_j S[e,j] * (me[e] @ final_w[:,j,:].T)     # [E, D]

S is computed without any scatter for the common case: the host bins each
edge's angles into rank slots (rank r = r-th angle of its edge) and lays out
a_sbf^T so that rank pass r streams through the PE aligned by edge; PSUM
accumulation over the rank passes IS the segment sum. Edges with more than
R0 angles spill into compacted overflow levels whose partial sums are
scatter-added (dma_scatter_add) with *unique* indices per call — duplicate
indices inside one scatter call race on the CCE read-modify-write path and
lose updates (measured), unique ones are exact.

Sharding: edges are contiguous, 25000 per core; angles are binned by owner
core (kj // 25000) so scatter indices fit int16 and no collective is needed.
"""

import numpy as np

import concourse.bass as bass
import concourse.mybir as mybir
import concourse.tile as tile
from concourse import bacc, bass_utils, library_config

F32 = mybir.dt.float32
I16 = mybir.dt.int16
AF = mybir.ActivationFunctionType
OP = mybir.AluOpType

D = 128
A_DIM = 49
N_RBF = 6
N_BIL = 8
N_CORES = 8
TRASH_ROW = 25_000      # scatter target for padding tokens (adds zeros)
MAX_SCATTER = 4096      # tokens per dma_scatter_add call (ring limit)


class Cfg:
    """levels: tuple of (cap_i, R_i); level 0 cap must equal e_pad."""

    def __init__(self, e_valid, e_pad, levels):
        self.e_valid = e_valid
        self.e_pad = e_pad
        self.levels = tuple(levels)
        assert e_pad % 2048 == 0
        assert levels[0][0] == e_pad
        for cap, r in levels:
            assert cap % 512 == 0 and r % 2 == 0
        self.n_chunks_b = e_pad // 1024
        # packed aT stream: one 512-col block per (level, edge-chunk, rank
        # pair); host pads the array to a 2048-col multiple.
        self.n_blocks = sum((cap // 512) * (r // 2) for cap, r in levels)
        self.at_cols = ((self.n_blocks * 512 + 2047) // 2048) * 2048

    def key(self):
        return (self.e_valid, self.e_pad, self.levels)


def build_nc(cfg: Cfg, phases=(1, 2)):
    nc = bacc.Bacc(None)
    EP = cfg.e_pad

    aT = nc.dram_tensor("a_t", [64 + A_DIM, cfg.at_cols], F32,
                        kind="ExternalInput")
    mjiT = nc.dram_tensor("mji_t", [D, EP], F32, kind="ExternalInput")
    erbf = nc.dram_tensor("erbf_t", [N_RBF, EP], F32, kind="ExternalInput")
    wnbr = nc.dram_tensor("w_nbr", [D, D], F32, kind="ExternalInput")
    bnbr = nc.dram_tensor("b_nbr", [D, 1], F32, kind="ExternalInput")
    wes = nc.dram_tensor("w_e", [N_RBF, D], F32, kind="ExternalInput")
    wa2 = nc.dram_tensor("w_a2", [64 + A_DIM, N_BIL], F32,
                         kind="ExternalInput")
    i8d = nc.dram_tensor("i8", [N_BIL, N_BIL], F32, kind="ExternalInput")
    t2 = nc.dram_tensor("t2", [D, N_BIL * D], F32, kind="ExternalInput")
    idxd = {}
    for li, (cap, _r) in enumerate(cfg.levels):
        if li == 0:
            continue
        idxd[li] = nc.dram_tensor(f"idx_l{li}", [128, cap // 16], I16,
                                  kind="ExternalInput")
    outd = nc.dram_tensor("out", [EP, D], F32, kind="ExternalOutput")
    sovf = nc.dram_tensor("s_ovf", [EP, 64], F32)   # internal, 256B rows

    n_groups0 = EP // 128

    with tile.TileContext(nc) as tc:
        nc.gpsimd.load_library(library_config.mlp)
        with tc.tile_pool(name="const", bufs=1) as cp:
            wa_sb = cp.tile([64 + A_DIM, N_BIL], F32)
            nc.sync.dma_start(out=wa_sb[:], in_=wa2[:])
            i8_sb = cp.tile([N_BIL, N_BIL], F32)
            nc.sync.dma_start(out=i8_sb[:], in_=i8d[:])
            s_sbuf = cp.tile([128, n_groups0 * N_BIL], F32)

            # ---- zero the overflow accumulator ----
            s_flat = sovf.ap().rearrange("(p x) c -> p (x c)", p=128)
            zcols = s_flat.shape[1]
            with tc.tile_pool(name="zero", bufs=1) as zp:
                zt = zp.tile([128, zcols // 4], F32)
                nc.vector.memset(zt[:], 0.0)
                for q in range(4):
                    nc.sync.dma_start(
                        out=s_flat[:, q * (zcols // 4):(q + 1) * (zcols // 4)],
                        in_=zt[:])

            # ============ phase A: S via rank-pass PSUM accumulation =======
            if 1 not in phases:
                nc.vector.memset(s_sbuf[:], 0.0)
            if 1 in phases:
              with tc.tile_pool(name="pa", bufs=3) as pa, \
                 tc.tile_pool(name="stp", bufs=2) as stp, \
                 tc.tile_pool(name="stage", bufs=1) as stage, \
                 tc.tile_pool(name="pss", bufs=2, space="PSUM") as pss, \
                 tc.tile_pool(name="pst", bufs=2, space="PSUM") as pst:
                at_tiles = {}

                def at_block(b):
                    ck = b // 4
                    if ck not in at_tiles:
                        t = pa.tile([64 + A_DIM, 2048], F32, tag="at")
                        nc.sync.dma_start(
                            out=t[:], in_=aT[:, ck * 2048:(ck + 1) * 2048])
                        at_tiles.clear()
                        at_tiles[ck] = t
                    off = (b % 4) * 512
                    return at_tiles[ck][:, off:off + 512]

                stages = {}
                idx_sb = {}
                blk = 0
                for li, (cap, R) in enumerate(cfg.levels):
                    n_groups = cap // 128
                    if li > 0:
                        stages[li] = stage.tile([128, n_groups * N_BIL], F32,
                                                tag=f"stage{li}",
                                                name=f"stage{li}")
                        idx_sb[li] = stage.tile([128, cap // 16], I16,
                                                tag=f"idx{li}",
                                                name=f"idx{li}")
                        nc.sync.dma_start(out=idx_sb[li][:], in_=idxd[li][:])
                    pt = None
                    for c in range(cap // 512):
                        # A psum accumulation group must keep one lhsT
                        # partition base (base switches mid-group wedge the
                        # PE): even ranks (base 0) and odd ranks (base 64)
                        # accumulate separately, merged on DVE.
                        abs_ = []
                        for p in range(R // 2):
                            abs_.append(at_block(blk))
                            blk += 1
                        ps_e = pss.tile([N_BIL, 512], F32, tag="se")
                        for p in range(R // 2):
                            nc.tensor.matmul(
                                ps_e[:], wa_sb[0:A_DIM, :],
                                abs_[p][0:A_DIM, :],
                                start=(p == 0), stop=(p == R // 2 - 1))
                        ps_o = pss.tile([N_BIL, 512], F32, tag="so")
                        for p in range(R // 2):
                            nc.tensor.matmul(
                                ps_o[:], wa_sb[64:64 + A_DIM, :],
                                abs_[p][64:64 + A_DIM, :],
                                start=(p == 0), stop=(p == R // 2 - 1))
                        st = stp.tile([N_BIL, 512], F32, tag="st")
                        nc.vector.tensor_copy(out=st[:], in_=ps_e[:])
                        nc.vector.tensor_add(st[:], st[:], ps_o[:])
                        # transpose [8,128] pieces -> [128,8] psum slots
                        for q in range(4):
                            gl = c * 4 + q
                            slot = gl % 64
                            if slot == 0:
                                pt = pst.tile([128, 512], F32, tag="tp")
                            nc.tensor.matmul(
                                pt[:, slot * 8:(slot + 1) * 8],
                                st[:, q * 128:(q + 1) * 128],
                                i8_sb[:], start=True, stop=True)
                            if slot == 63 or gl == n_groups - 1:
                                g0 = gl - slot
                                dst = s_sbuf if li == 0 else stages[li]
                                nc.vector.tensor_copy(
                                    out=dst[:, g0 * 8:(gl + 1) * 8],
                                    in_=pt[:, :(slot + 1) * 8])
                    # overflow scatter (unique indices per call)
                    if li > 0:
                        t0 = 0
                        while t0 < cap:
                            n_tok = min(MAX_SCATTER, cap - t0)
                            in_ap = stages[li][:, t0 // 128 * 8:
                                               (t0 + n_tok) // 128 * 8]
                            nc.gpsimd.dma_scatter_add(
                                out_ap=sovf[:, 0:N_BIL],
                                in_ap=in_ap.rearrange("p (c e) -> p c e",
                                                      e=N_BIL),
                                idxs_ap=idx_sb[li][:, t0 // 16:
                                                   (t0 + n_tok) // 16],
                                num_idxs=n_tok,
                                num_idxs_reg=n_tok,
                                elem_size=N_BIL,
                                elem_step=64,
                                queue_num=0)
                            t0 += n_tok

            # ============ phase B: edge transform + S apply ================
            if 2 not in phases:
                with tc.tile_pool(name="dbg", bufs=1) as dbg:
                    dtile = dbg.tile([128, n_groups0 * N_BIL], F32)
                    nc.vector.tensor_copy(out=dtile[:], in_=s_sbuf[:])
                    ov = outd.ap().rearrange("(p x) c -> p (x c)", p=128)
                    nc.sync.dma_start(out=ov[:, 0:n_groups0 * N_BIL],
                                      in_=dtile[:])
            if 2 in phases:
              wn_sb = cp.tile([D, D], F32)
              nc.sync.dma_start(out=wn_sb[:], in_=wnbr[:])
              b_sb = cp.tile([D, 1], F32)
              nc.sync.dma_start(out=b_sb[:], in_=bnbr[:])
              we_sb = cp.tile([N_RBF, D], F32)
              nc.sync.dma_start(out=we_sb[:], in_=wes[:])
              t2_sb = cp.tile([D, N_BIL * D], F32)
              nc.sync.dma_start(out=t2_sb[:], in_=t2[:])

              s_view = sovf.ap().rearrange("(t p) c -> p t c", p=128)

              with tc.tile_pool(name="pb", bufs=2) as pb, \
                   tc.tile_pool(name="sbp", bufs=2) as sbp, \
                   tc.tile_pool(name="accp", bufs=3) as accp, \
                   tc.tile_pool(name="psmm", bufs=2, space="PSUM") as pmm, \
                   tc.tile_pool(name="psy", bufs=2, space="PSUM") as py:
                  for c in range(cfg.n_chunks_b):
                      er_sb = pb.tile([N_RBF, 1024], F32, tag="er")
                      nc.sync.dma_start(out=er_sb[:],
                                        in_=erbf[:, c * 1024:(c + 1) * 1024])
                      te_ps = pmm.tile([128, 1024], F32, tag="mm")
                      for n in range(2):
                          nc.tensor.matmul(
                              te_ps[:, n * 512:(n + 1) * 512],
                              we_sb[:], er_sb[:, n * 512:(n + 1) * 512],
                              start=True, stop=True)
                      mj = pb.tile([128, 1024], F32, tag="mj")
                      nc.sync.dma_start(out=mj[:],
                                        in_=mjiT[:, c * 1024:(c + 1) * 1024])
                      tm_ps = pmm.tile([128, 1024], F32, tag="mm")
                      for n in range(2):
                          nc.tensor.matmul(
                              tm_ps[:, n * 512:(n + 1) * 512],
                              wn_sb[:], mj[:, n * 512:(n + 1) * 512],
                              start=True, stop=True)
                      sg_sb = pb.tile([128, 1024], F32, tag="sg")
                      nc.scalar.activation(sg_sb[:], tm_ps[:], AF.Sigmoid,
                                           bias=b_sb[:, 0:1])
                      tm_sb = pb.tile([128, 1024], F32, tag="tm")
                      # silu(x+b) = (x+b) * sigmoid(x+b)
                      nc.vector.scalar_tensor_tensor(
                          out=tm_sb[:], in0=tm_ps[:], scalar=b_sb[:, 0:1],
                          in1=sg_sb[:], op0=OP.add, op1=OP.mult)
                      me_sb = pb.tile([128, 1024], F32, tag="me")
                      nc.vector.tensor_mul(me_sb[:], tm_sb[:], te_ps[:])

                      so_sb = sbp.tile([128, 8 * 64], F32, tag="so")
                      nc.sync.dma_start(out=so_sb[:],
                                        in_=s_view[:, c * 8:(c + 1) * 8, :])
                      s_tot = sbp.tile([128, 64], F32, tag="stot")
                      nc.vector.tensor_add(
                          s_tot[:].rearrange("p (t j) -> p t j", j=8),
                          s_sbuf[:, c * 64:(c + 1) * 64]
                          .rearrange("p (t j) -> p t j", j=8),
                          so_sb[:].rearrange("p (t j) -> p t j", j=64)
                          [:, :, 0:8])

                      for tt in range(8):
                          y = py.tile([128, N_BIL * D], F32, tag="y")
                          lhsT = me_sb[:, tt * 128:(tt + 1) * 128]
                          nc.tensor.matmul(y[:, 0:512], lhsT, t2_sb[:, 0:512],
                                           start=True, stop=True)
                          nc.tensor.matmul(y[:, 512:1024], lhsT,
                                           t2_sb[:, 512:1024],
                                           start=True, stop=True)
                          acc = accp.tile([128, D], F32, tag="acc")
                          nc.vector.tensor_scalar_mul(
                              acc[:], y[:, 0:D], s_tot[:, tt * 8:tt * 8 + 1])
                          for j in range(1, N_BIL):
                              nc.vector.scalar_tensor_tensor(
                                  out=acc[:],
                                  in0=y[:, j * D:(j + 1) * D],
                                  scalar=s_tot[:, tt * 8 + j:tt * 8 + j + 1],
                                  in1=acc[:],
                                  op0=OP.mult, op1=OP.add)
                          e0 = (c * 8 + tt) * 128
                          nc.sync.dma_start(out=outd[e0:e0 + 128, :], in_=acc[:])
    nc.finalize()
    return nc


# ----------------------------------------------------------------------------
# host-side sharding / unsharding
# ----------------------------------------------------------------------------

def make_cfg(kj, n_edges, ev=25_000, ep=26_624):
    n_cores = (n_edges + ev - 1) // ev
    owner = np.minimum(kj // ev, n_cores - 1)
    caps = []  # per level >=1: max count over cores
    max_rank = 0
    for c in range(n_cores):
        loc = kj[owner == c] - c * ev
        cnt = np.bincount(loc, minlength=ev)
        max_rank = max(max_rank, int(cnt.max()))
        base = 4
        li = 0
        while (cnt > base).any():
            n = int((cnt > base).sum())
            if li >= len(caps):
                caps.append(n)
            else:
                caps[li] = max(caps[li], n)
            base += 4
            li += 1
    levels = [(ep, 4)]
    for n in caps:
        levels.append((max(512, ((n + 511) // 512) * 512), 4))
    return Cfg(ev, ep, levels)


def prep_in_maps(cfg: Cfg, m_ji, nbr_list, angle_list, e_rbf, a_sbf, kj_idx,
                 W_nbr, b_nbr, W_e, W_a, final_w):
    del nbr_list, angle_list
    m_ji = np.asarray(m_ji, np.float32)
    e_rbf = np.asarray(e_rbf, np.float32)
    a_sbf = np.asarray(a_sbf, np.float32)
    kj = np.asarray(kj_idx).astype(np.int64)
    W_nbr = np.asarray(W_nbr, np.float32)
    b_nbr = np.asarray(b_nbr, np.float32)
    W_e = np.asarray(W_e, np.float32)
    W_a = np.asarray(W_a, np.float32)
    final_w = np.asarray(final_w, np.float32)

    n_edges = m_ji.shape[0]
    ev = cfg.e_valid
    ep = cfg.e_pad
    n_cores = (n_edges + ev - 1) // ev
    owner = np.minimum(kj // ev, n_cores - 1)

    wa2 = np.zeros((64 + A_DIM, N_BIL), np.float32)
    wa2[0:A_DIM] = W_a
    wa2[64:64 + A_DIM] = W_a
    t2 = np.ascontiguousarray(final_w.transpose(2, 1, 0).reshape(D, N_BIL * D))
    bn = np.ascontiguousarray(b_nbr.reshape(D, 1))
    i8 = np.eye(N_BIL, dtype=np.float32)

    in_maps = []
    for c in range(n_cores):
        sel = np.nonzero(owner == c)[0]
        loc = kj[sel] - c * ev
        order = np.argsort(loc, kind="stable")
        loc = loc[order]
        rows = sel[order]                       # a_sbf row per sorted token
        cnt = np.bincount(loc, minlength=ep)
        starts = np.concatenate([[0], np.cumsum(cnt)])

        # pack the rank-pass stream
        at = np.zeros((64 + A_DIM, cfg.at_cols), np.float32)
        col = 0
        base = 0
        idx_maps = {}
        for li, (cap, R) in enumerate(cfg.levels):
            if li == 0:
                elist = np.arange(ep)
            else:
                elist = np.nonzero(cnt > base)[0]
                assert len(elist) <= cap, (li, len(elist), cap)
                el_pad = np.full(cap, cfg.e_valid, np.int64)
                el_pad[:len(elist)] = elist
                w16 = el_pad.astype(np.int16).reshape(-1, 16).T
                idx_maps[f"idx_l{li}"] = np.ascontiguousarray(
                    np.tile(w16, (8, 1)))
            # A_r [cap, 49] per rank
            a_rs = []
            for r in range(R):
                a_r = np.zeros((cap, A_DIM), np.float32)
                has = np.nonzero(cnt[elist] > base + r)[0]  # pos within elist
                tok = starts[elist[has]] + base + r
                a_r[has] = a_sbf[rows[tok]]
                a_rs.append(a_r)
            for cc in range(cap // 512):
                for p in range(R // 2):
                    at[0:A_DIM, col:col + 512] = \
                        a_rs[2 * p][cc * 512:(cc + 1) * 512].T
                    at[64:64 + A_DIM, col:col + 512] = \
                        a_rs[2 * p + 1][cc * 512:(cc + 1) * 512].T
                    col += 512
            base += R
        assert int(cnt.max()) <= base, "levels do not cover max multiplicity"

        e0, e1 = c * ev, min((c + 1) * ev, n_edges)
        mjiT = np.zeros((D, ep), np.float32)
        mjiT[:, :e1 - e0] = m_ji[e0:e1].T
        erbfT = np.zeros((N_RBF, ep), np.float32)
        erbfT[:, :e1 - e0] = e_rbf[e0:e1].T

        im = {
            "a_t": at, "mji_t": np.ascontiguousarray(mjiT),
            "erbf_t": erbfT, "w_nbr": W_nbr, "b_nbr": bn,
            "w_e": W_e, "w_a2": wa2, "i8": i8, "t2": t2,
        }
        im.update(idx_maps)
        in_maps.append(im)
    return in_maps


def gather_output(cfg: Cfg, results, n_edges):
    outs = []
    ev = cfg.e_valid
    for c, r in enumerate(results):
        e0, e1 = c * ev, min((c + 1) * ev, n_edges)
        outs.append(np.asarray(r["out"])[:e1 - e0])
    return np.ascontiguousarray(np.concatenate(outs, axis=0))


_NC_CACHE = {}


def run_on_hw(inputs, cfg=None, trace=False, trace_cores=None):
    kj = np.asarray(inputs["kj_idx"]).astype(np.int64)
    if cfg is None:
        cfg = make_cfg(kj, inputs["m_ji"].shape[0])
    key = cfg.key()
    if key not in _NC_CACHE:
        _NC_CACHE[key] = build_nc(cfg)
    nc = _NC_CACHE[key]
    in_maps = prep_in_maps(cfg, **inputs)
    res = bass_utils.run_bass_kernel_spmd(
        nc, in_maps, core_ids=list(range(len(in_maps))),
        trace=trace, trace_cores=trace_cores)
    out = gather_output(cfg, res.results, inputs["m_ji"].shape[0])
    return out, res


def kernel(**inputs) -> np.ndarray:
    out, _ = run_on_hw(inputs)
    return out



# revision 2
# speedup vs baseline: 3.9668x; 3.9668x over previous
"""Trainium2 Bass kernel for the DimeNet-style directed-message block.

Reference computation (W = n_angles, E = n_edges, D = 128, A = 49, J = 8):
    m_kj     = m_ji[kj_idx]                          # [W, D]
    transf_m = silu(m_kj @ W_nbr + b_nbr)            # [W, D]
    transf_e = e_rbf[kj_idx] @ W_e                   # [W, D]
    m_and_e  = transf_m * transf_e                   # [W, D]
    transf_a = a_sbf @ W_a                           # [W, J]
    out[w,i] = sum_{j,l} transf_a[w,j] m_and_e[w,l] final_w[i,j,l]
    final    = segment_sum(out, kj_idx, E)           # [E, D]

Algebraic refactor: every per-angle factor except transf_a depends on the
angle only through kj_idx, so the segment sum commutes through the bilinear
form:
    me       = silu(m_ji @ W_nbr + b) * (e_rbf @ W_e)        # [E, D]
    S        = segment_sum(a_sbf @ W_a, kj_idx, E)           # [E, J]
    final[e] = sum_j S[e,j] * (me[e] @ final_w[:,j,:].T)     # [E, D]

S without any scatter: edges are sharded contiguously (25000 per core,
angles binned by owner core kj // 25000), then *permuted within the core by
descending angle multiplicity*. Groups of 128 edge slots get a static
rank-pair count rg[g] (max over cores of the group's max multiplicity,
halved); the host lays the angles out as [98, 128]-blocks (rank 2p in
partitions 0:49, rank 2p+1 in 49:98) so that

    S[group g] = sum_p  aT_block(g,p).T @ [W_a; W_a]         # [128, 8]

is a plain PSUM accumulation with the a-stream as the (bf16, FWL-fast)
stationary operand.  Descending sort makes the rank profile a staircase, so
padding is only ~12% and there is no overflow level and no scatter at all.

Phase B consumes S edge-major: per 128-edge tile, y = me.T_chunk @ t2 gives
the eight bilinear products [128, 8*128] and the j-combination is
tensor_scalar chains with S columns as per-partition scalars, split between
the scalar engine (PSUM->SBUF bf16 copy) and DVE (scaled accumulation).
"""

import numpy as np

import concourse.bass as bass
import concourse.mybir as mybir
import concourse.tile as tile
from concourse import bacc, bass_utils

F32 = mybir.dt.float32
BF16 = mybir.dt.bfloat16
AF = mybir.ActivationFunctionType
OP = mybir.AluOpType

D = 128
A_DIM = 49
N_RBF = 6
N_BIL = 8
N_CORES = 8
AT_P = 2 * A_DIM          # 98 partitions: even rank 0:49, odd rank 49:98
AT_TILE = 4096            # aT stream tile width (cols); 32 blocks per tile


class Cfg:
    def __init__(self, e_valid, e_pad, rg):
        self.e_valid = e_valid
        self.e_pad = e_pad
        self.rg = tuple(int(r) for r in rg)      # rank-pairs per 128-edge group
        assert e_pad % 1024 == 0
        self.n_groups = e_pad // 128
        assert len(self.rg) == self.n_groups
        self.n_blocks = sum(self.rg)
        self.at_cols = ((self.n_blocks * 128 + AT_TILE - 1) // AT_TILE) * AT_TILE
        self.n_chunks = e_pad // 1024

    def key(self):
        return (self.e_valid, self.e_pad, self.rg)


def build_nc(cfg: Cfg, phases=(1, 2)):
    nc = bacc.Bacc(None)
    EP = cfg.e_pad
    NG = cfg.n_groups

    aT = nc.dram_tensor("a_t", [AT_P, cfg.at_cols], BF16, kind="ExternalInput")
    mjiT = nc.dram_tensor("mji_t", [D, EP], BF16, kind="ExternalInput")
    erbf = nc.dram_tensor("erbf_t", [N_RBF, EP], BF16, kind="ExternalInput")
    wnbr = nc.dram_tensor("w_nbr", [D, D], BF16, kind="ExternalInput")
    bnbr = nc.dram_tensor("b_nbr", [D, 1], F32, kind="ExternalInput")
    wes = nc.dram_tensor("w_e", [N_RBF, D], BF16, kind="ExternalInput")
    wa2 = nc.dram_tensor("w_a2", [AT_P, N_BIL], BF16, kind="ExternalInput")
    t2 = nc.dram_tensor("t2", [D, N_BIL * D], BF16, kind="ExternalInput")
    outd = nc.dram_tensor("out", [EP, D], F32, kind="ExternalOutput")

    with tile.TileContext(nc) as tc:
        with tc.tile_pool(name="const", bufs=1) as cp:
            wa_sb = cp.tile([AT_P, N_BIL], BF16)
            nc.sync.dma_start(out=wa_sb[:], in_=wa2[:])
            s_sbuf = cp.tile([128, NG * N_BIL], F32)
            nc.vector.memset(s_sbuf[:], 0.0)

            # ============ phase A: S via per-group PSUM rank accumulation ===
            if 1 in phases:
              with tc.tile_pool(name="pa", bufs=3) as pa, \
                   tc.tile_pool(name="pss", bufs=2, space="PSUM") as pss:
                at_tiles = {}

                def at_block(b):
                    tk = b // (AT_TILE // 128)
                    if tk not in at_tiles:
                        t = pa.tile([AT_P, AT_TILE], BF16, tag="at")
                        nc.sync.dma_start(
                            out=t[:], in_=aT[:, tk * AT_TILE:(tk + 1) * AT_TILE])
                        at_tiles.clear()
                        at_tiles[tk] = t
                    off = (b % (AT_TILE // 128)) * 128
                    return at_tiles[tk][:, off:off + 128]

                blk = 0
                for g0 in range(0, NG, 64):
                    g1 = min(g0 + 64, NG)
                    nz = sum(1 for g in range(g0, g1) if cfg.rg[g] > 0)
                    if nz == 0:
                        continue
                    ps = pss.tile([128, 512], F32, tag="ps")
                    for g in range(g0, g1):
                        R = cfg.rg[g]
                        if R == 0:
                            continue
                        sl = (g - g0) * N_BIL
                        for p in range(R):
                            nc.tensor.matmul(
                                ps[:, sl:sl + N_BIL], at_block(blk), wa_sb[:],
                                start=(p == 0), stop=(p == R - 1))
                            blk += 1
                    nc.vector.tensor_copy(
                        out=s_sbuf[:, g0 * N_BIL:(g0 + nz) * N_BIL],
                        in_=ps[:, :nz * N_BIL])

            # ============ phase B: edge transform + S apply ================
            if 2 in phases:
              wn_sb = cp.tile([D, D], BF16)
              nc.sync.dma_start(out=wn_sb[:], in_=wnbr[:])
              b_sb = cp.tile([D, 1], F32)
              nc.sync.dma_start(out=b_sb[:], in_=bnbr[:])
              we_sb = cp.tile([N_RBF, D], BF16)
              nc.sync.dma_start(out=we_sb[:], in_=wes[:])
              t2_sb = cp.tile([D, N_BIL * D], BF16)
              nc.sync.dma_start(out=t2_sb[:], in_=t2[:])

              with tc.tile_pool(name="pb", bufs=2) as pb, \
                   tc.tile_pool(name="accp", bufs=4) as accp, \
                   tc.tile_pool(name="ysb", bufs=2) as ysb, \
                   tc.tile_pool(name="psmm", bufs=2, space="PSUM") as pmm, \
                   tc.tile_pool(name="psy", bufs=2, space="PSUM") as py:
                  for c in range(cfg.n_chunks):
                      er_sb = pb.tile([N_RBF, 1024], BF16, tag="er")
                      nc.sync.dma_start(out=er_sb[:],
                                        in_=erbf[:, c * 1024:(c + 1) * 1024])
                      te_ps = pmm.tile([128, 1024], F32, tag="mm")
                      for n in range(2):
                          nc.tensor.matmul(
                              te_ps[:, n * 512:(n + 1) * 512],
                              we_sb[:], er_sb[:, n * 512:(n + 1) * 512],
                              start=True, stop=True)
                      mj = pb.tile([128, 1024], BF16, tag="mj")
                      nc.sync.dma_start(out=mj[:],
                                        in_=mjiT[:, c * 1024:(c + 1) * 1024])
                      tm_ps = pmm.tile([128, 1024], F32, tag="mm")
                      for n in range(2):
                          nc.tensor.matmul(
                              tm_ps[:, n * 512:(n + 1) * 512],
                              wn_sb[:], mj[:, n * 512:(n + 1) * 512],
                              start=True, stop=True)
                      tm_sb = pb.tile([128, 1024], F32, tag="tm")
                      nc.scalar.activation(tm_sb[:], tm_ps[:], AF.Silu,
                                           bias=b_sb[:, 0:1])
                      me_sb = pb.tile([128, 1024], BF16, tag="me")
                      nc.vector.tensor_mul(me_sb[:], tm_sb[:], te_ps[:])

                      for tt in range(8):
                          y = py.tile([128, N_BIL * D], F32, tag="y")
                          lhsT = me_sb[:, tt * 128:(tt + 1) * 128]
                          nc.tensor.matmul(y[:, 0:512], lhsT, t2_sb[:, 0:512],
                                           start=True, stop=True)
                          nc.tensor.matmul(y[:, 512:1024], lhsT,
                                           t2_sb[:, 512:1024],
                                           start=True, stop=True)
                          y_sb = ysb.tile([128, N_BIL * D], BF16, tag="ysb")
                          nc.scalar.activation(y_sb[:], y[:], AF.Copy)
                          g8 = (c * 8 + tt) * N_BIL
                          acc = accp.tile([128, D], BF16, tag="acc")
                          nc.vector.tensor_scalar_mul(
                              acc[:], y_sb[:, 0:D], s_sbuf[:, g8:g8 + 1])
                          for j in range(1, N_BIL - 1):
                              nc.vector.scalar_tensor_tensor(
                                  out=acc[:],
                                  in0=y_sb[:, j * D:(j + 1) * D],
                                  scalar=s_sbuf[:, g8 + j:g8 + j + 1],
                                  in1=acc[:],
                                  op0=OP.mult, op1=OP.add)
                          accf = accp.tile([128, D], F32, tag="accf")
                          nc.vector.scalar_tensor_tensor(
                              out=accf[:],
                              in0=y_sb[:, (N_BIL - 1) * D:N_BIL * D],
                              scalar=s_sbuf[:, g8 + N_BIL - 1:g8 + N_BIL],
                              in1=acc[:],
                              op0=OP.mult, op1=OP.add)
                          e0 = (c * 8 + tt) * 128
                          nc.sync.dma_start(out=outd[e0:e0 + 128, :],
                                            in_=accf[:])
            if 2 not in phases:
                ov = outd.ap().rearrange("(p x) c -> p (x c)", p=128)
                nc.sync.dma_start(out=ov[:, 0:NG * N_BIL], in_=s_sbuf[:])
    nc.finalize()
    return nc


# ----------------------------------------------------------------------------
# host-side sharding / unsharding
# ----------------------------------------------------------------------------

def make_cfg(kj, n_edges, ev=25_000, ep=26_624):
    n_cores = (n_edges + ev - 1) // ev
    owner = np.minimum(kj // ev, n_cores - 1)
    ng = ep // 128
    rg = np.zeros(ng, np.int64)
    for c in range(n_cores):
        loc = kj[owner == c] - c * ev
        cnt = np.bincount(loc, minlength=ev)
        s = np.zeros(ep, np.int64)
        s[:ev] = np.sort(cnt)[::-1]
        gmax = s.reshape(ng, 128).max(axis=1)
        rg = np.maximum(rg, (gmax + 1) // 2)
    return Cfg(ev, ep, tuple(int(r) for r in rg))


def prep_in_maps(cfg: Cfg, m_ji, nbr_list, angle_list, e_rbf, a_sbf, kj_idx,
                 W_nbr, b_nbr, W_e, W_a, final_w):
    del nbr_list, angle_list
    m_ji = np.asarray(m_ji, np.float32)
    e_rbf = np.asarray(e_rbf, np.float32)
    a_sbf = np.asarray(a_sbf, np.float32)
    kj = np.asarray(kj_idx).astype(np.int64)
    W_nbr = np.asarray(W_nbr, np.float32)
    b_nbr = np.asarray(b_nbr, np.float32)
    W_e = np.asarray(W_e, np.float32)
    W_a = np.asarray(W_a, np.float32)
    final_w = np.asarray(final_w, np.float32)

    n_edges = m_ji.shape[0]
    ev = cfg.e_valid
    ep = cfg.e_pad
    n_cores = (n_edges + ev - 1) // ev
    owner = np.minimum(kj // ev, n_cores - 1)

    wa2 = np.zeros((AT_P, N_BIL), np.float32)
    wa2[0:A_DIM] = W_a
    wa2[A_DIM:2 * A_DIM] = W_a
    t2 = np.ascontiguousarray(final_w.transpose(2, 1, 0).reshape(D, N_BIL * D))
    bn = np.ascontiguousarray(b_nbr.reshape(D, 1))

    in_maps = []
    perms = []
    for c in range(n_cores):
        sel = np.nonzero(owner == c)[0]
        loc = kj[sel] - c * ev
        cnt = np.bincount(loc, minlength=ev)
        edge_order = np.argsort(-cnt, kind="stable")     # slot -> local edge
        slot_of_edge = np.empty(ev, np.int64)
        slot_of_edge[edge_order] = np.arange(ev)
        ang_slot = slot_of_edge[loc]
        order = np.argsort(ang_slot, kind="stable")
        rows = sel[order]                 # a_sbf row per (slot-sorted) token
        cnt_slot = np.bincount(ang_slot, minlength=ep)
        starts = np.concatenate([[0], np.cumsum(cnt_slot)])

        at = np.zeros((AT_P, cfg.at_cols), np.float32)
        col = 0
        for g in range(cfg.n_groups):
            sl = np.arange(g * 128, (g + 1) * 128)
            csl = cnt_slot[sl]
            for p in range(cfg.rg[g]):
                for half, r in ((0, 2 * p), (1, 2 * p + 1)):
                    has = np.nonzero(csl > r)[0]
                    if len(has):
                        tok = starts[sl[has]] + r
                        at[half * A_DIM:(half + 1) * A_DIM,
                           col + has] = a_sbf[rows[tok]].T
                col += 128
        assert col == cfg.n_blocks * 128

        e0, e1 = c * ev, min((c + 1) * ev, n_edges)
        mjiT = np.zeros((D, ep), np.float32)
        mjiT[:, :e1 - e0] = m_ji[e0:e1][edge_order[:e1 - e0]].T
        erbfT = np.zeros((N_RBF, ep), np.float32)
        erbfT[:, :e1 - e0] = e_rbf[e0:e1][edge_order[:e1 - e0]].T

        bf = mybir.dt.np(BF16)
        im = {
            "a_t": at.astype(bf), "mji_t": mjiT.astype(bf),
            "erbf_t": erbfT.astype(bf), "w_nbr": W_nbr.astype(bf),
            "b_nbr": bn, "w_e": W_e.astype(bf), "w_a2": wa2.astype(bf),
            "t2": t2.astype(bf),
        }
        in_maps.append(im)
        perms.append(edge_order)
    return in_maps, perms


def gather_output(cfg: Cfg, results, perms, n_edges):
    ev = cfg.e_valid
    out = np.empty((n_edges, D), np.float32)
    for c, r in enumerate(results):
        e0, e1 = c * ev, min((c + 1) * ev, n_edges)
        out[e0 + perms[c][:e1 - e0]] = np.asarray(r["out"])[:e1 - e0]
    return out


_NC_CACHE = {}


def run_on_hw(inputs, cfg=None, trace=False, trace_cores=None, phases=(1, 2)):
    kj = np.asarray(inputs["kj_idx"]).astype(np.int64)
    if cfg is None:
        cfg = make_cfg(kj, inputs["m_ji"].shape[0])
    key = (cfg.key(), phases)
    if key not in _NC_CACHE:
        _NC_CACHE[key] = build_nc(cfg, phases=phases)
    nc = _NC_CACHE[key]
    in_maps, perms = prep_in_maps(cfg, **inputs)
    res = bass_utils.run_bass_kernel_spmd(
        nc, in_maps, core_ids=list(range(len(in_maps))),
        trace=trace, trace_cores=trace_cores)
    out = gather_output(cfg, res.results, perms, inputs["m_ji"].shape[0])
    return out, res


def kernel(**inputs) -> np.ndarray:
    out, _ = run_on_hw(inputs)
    return out


# revision 5
# speedup vs baseline: 4.1338x; 1.0421x over previous
"""Trainium2 Bass kernel for the DimeNet-style directed-message block.

Reference computation (W = n_angles, E = n_edges, D = 128, A = 49, J = 8):
    m_kj     = m_ji[kj_idx]                          # [W, D]
    transf_m = silu(m_kj @ W_nbr + b_nbr)            # [W, D]
    transf_e = e_rbf[kj_idx] @ W_e                   # [W, D]
    m_and_e  = transf_m * transf_e                   # [W, D]
    transf_a = a_sbf @ W_a                           # [W, J]
    out[w,i] = sum_{j,l} transf_a[w,j] m_and_e[w,l] final_w[i,j,l]
    final    = segment_sum(out, kj_idx, E)           # [E, D]

Algebraic refactor: every per-angle factor except transf_a depends on the
angle only through kj_idx, so the segment sum commutes through the bilinear
form:
    me       = silu(m_ji @ W_nbr + b) * (e_rbf @ W_e)        # [E, D]
    S        = segment_sum(a_sbf @ W_a, kj_idx, E)           # [E, J]
    final[e] = sum_j S[e,j] * (me[e] @ final_w[:,j,:].T)     # [E, D]

S without any scatter: edges are sharded contiguously (25000 per core,
angles binned by owner core kj // 25000), then *permuted within the core by
descending angle multiplicity*. Groups of 128 edge slots get a static
rank-pair count rg[g] (max over cores of the group's max multiplicity,
halved); the host lays the angles out as [98, 128]-blocks (rank 2p in
partitions 0:49, rank 2p+1 in 49:98) so that

    S[group g] = sum_p  aT_block(g,p).T @ [W_a; W_a]         # [128, 8]

is a plain PSUM accumulation with the a-stream as the (bf16, FWL-fast)
stationary operand.  Descending sort makes the rank profile a staircase, so
padding is only ~12% and there is no overflow level and no scatter at all.

Phase B consumes S edge-major: per 128-edge tile, y = me.T_chunk @ t2 gives
the eight bilinear products [128, 8*128] and the j-combination is
tensor_scalar chains with S columns as per-partition scalars, split between
the scalar engine (PSUM->SBUF bf16 copy) and DVE (scaled accumulation).
"""

import numpy as np

import concourse.bass as bass
import concourse.mybir as mybir
import concourse.tile as tile
from concourse import bacc, bass_utils

F32 = mybir.dt.float32
BF16 = mybir.dt.bfloat16
AF = mybir.ActivationFunctionType
OP = mybir.AluOpType

D = 128
A_DIM = 49
N_RBF = 6
N_BIL = 8
N_CORES = 8
AT_P = 2 * A_DIM          # 98 partitions: even rank 0:49, odd rank 49:98
AT_TILE = 4096            # aT stream tile width (cols); 32 blocks per tile


class Cfg:
    def __init__(self, e_valid, e_pad, rg):
        self.e_valid = e_valid
        self.e_pad = e_pad
        self.rg = tuple(int(r) for r in rg)      # rank-pairs per 128-edge group
        assert e_pad % 1024 == 0
        self.n_groups = e_pad // 128
        assert len(self.rg) == self.n_groups
        self.n_blocks = sum(self.rg)
        self.at_cols = ((self.n_blocks * 128 + AT_TILE - 1) // AT_TILE) * AT_TILE
        self.n_chunks = e_pad // 1024

    def key(self):
        return (self.e_valid, self.e_pad, self.rg)


def build_nc(cfg: Cfg, phases=(1, 2)):
    nc = bacc.Bacc(None)
    EP = cfg.e_pad
    NG = cfg.n_groups

    aT = nc.dram_tensor("a_t", [AT_P, cfg.at_cols], BF16, kind="ExternalInput")
    mjiT = nc.dram_tensor("mji_t", [D, EP], BF16, kind="ExternalInput")
    erbf = nc.dram_tensor("erbf_t", [N_RBF, EP], BF16, kind="ExternalInput")
    wnbr = nc.dram_tensor("w_nbr", [D, D], BF16, kind="ExternalInput")
    bnbr = nc.dram_tensor("b_nbr", [D, 1], F32, kind="ExternalInput")
    wes = nc.dram_tensor("w_e", [N_RBF, D], BF16, kind="ExternalInput")
    wa2 = nc.dram_tensor("w_a2", [AT_P, N_BIL], BF16, kind="ExternalInput")
    t2 = nc.dram_tensor("t2", [D, N_BIL * D], BF16, kind="ExternalInput")
    outd = nc.dram_tensor("out", [EP, D], F32, kind="ExternalOutput")

    with tile.TileContext(nc) as tc:
        with tc.tile_pool(name="const", bufs=1) as cp:
            wa_sb = cp.tile([AT_P, N_BIL], BF16)
            nc.sync.dma_start(out=wa_sb[:], in_=wa2[:])
            s_sbuf = cp.tile([128, NG * N_BIL], F32)
            nc.vector.memset(s_sbuf[:], 0.0)

            # ============ phase A: S via per-group PSUM rank accumulation ===
            if 1 in phases:
              with tc.tile_pool(name="pa", bufs=3) as pa, \
                   tc.tile_pool(name="pss", bufs=2, space="PSUM") as pss:
                at_tiles = {}

                def at_block(b):
                    tk = b // (AT_TILE // 128)
                    if tk not in at_tiles:
                        t = pa.tile([AT_P, AT_TILE], BF16, tag="at")
                        nc.sync.dma_start(
                            out=t[:], in_=aT[:, tk * AT_TILE:(tk + 1) * AT_TILE])
                        at_tiles.clear()
                        at_tiles[tk] = t
                    off = (b % (AT_TILE // 128)) * 128
                    return at_tiles[tk][:, off:off + 128]

                blk = 0
                for g0 in range(0, NG, 64):
                    g1 = min(g0 + 64, NG)
                    nz = sum(1 for g in range(g0, g1) if cfg.rg[g] > 0)
                    if nz == 0:
                        continue
                    ps = pss.tile([128, 512], F32, tag="ps")
                    for g in range(g0, g1):
                        R = cfg.rg[g]
                        if R == 0:
                            continue
                        sl = (g - g0) * N_BIL
                        for p in range(R):
                            nc.tensor.matmul(
                                ps[:, sl:sl + N_BIL], at_block(blk), wa_sb[:],
                                start=(p == 0), stop=(p == R - 1))
                            blk += 1
                    nc.vector.tensor_copy(
                        out=s_sbuf[:, g0 * N_BIL:(g0 + nz) * N_BIL],
                        in_=ps[:, :nz * N_BIL])

            # ============ phase B: edge transform + S apply ================
            if 2 in phases:
              wn_sb = cp.tile([D, D], BF16)
              nc.sync.dma_start(out=wn_sb[:], in_=wnbr[:])
              b_sb = cp.tile([D, 1], F32)
              nc.sync.dma_start(out=b_sb[:], in_=bnbr[:])
              we_sb = cp.tile([N_RBF, D], BF16)
              nc.sync.dma_start(out=we_sb[:], in_=wes[:])
              t2_sb = cp.tile([D, N_BIL * D], BF16)
              nc.sync.dma_start(out=t2_sb[:], in_=t2[:])

              ov = outd.ap().rearrange("(x p) i -> p x i", p=128)
              with tc.tile_pool(name="pb", bufs=2) as pb, \
                   tc.tile_pool(name="zcp", bufs=2) as zcp, \
                   tc.tile_pool(name="trp", bufs=2) as trp, \
                   tc.tile_pool(name="psmm", bufs=2, space="PSUM") as pmm, \
                   tc.tile_pool(name="psy", bufs=2, space="PSUM") as py:
                  for c in range(cfg.n_chunks):
                      er_sb = pb.tile([N_RBF, 1024], BF16, tag="er")
                      nc.sync.dma_start(out=er_sb[:],
                                        in_=erbf[:, c * 1024:(c + 1) * 1024])
                      te_ps = pmm.tile([128, 1024], F32, tag="mm")
                      for n in range(2):
                          nc.tensor.matmul(
                              te_ps[:, n * 512:(n + 1) * 512],
                              we_sb[:], er_sb[:, n * 512:(n + 1) * 512],
                              start=True, stop=True)
                      mj = pb.tile([128, 1024], BF16, tag="mj")
                      nc.sync.dma_start(out=mj[:],
                                        in_=mjiT[:, c * 1024:(c + 1) * 1024])
                      tm_ps = pmm.tile([128, 1024], F32, tag="mm")
                      for n in range(2):
                          nc.tensor.matmul(
                              tm_ps[:, n * 512:(n + 1) * 512],
                              wn_sb[:], mj[:, n * 512:(n + 1) * 512],
                              start=True, stop=True)
                      tm_sb = pb.tile([128, 1024], F32, tag="tm")
                      nc.scalar.activation(tm_sb[:], tm_ps[:], AF.Silu,
                                           bias=b_sb[:, 0:1])
                      me_sb = pb.tile([128, 1024], BF16, tag="me")
                      nc.vector.tensor_mul(me_sb[:], tm_sb[:], te_ps[:])

                      # z[p, tt, j, i] = S[p, (c,tt),j] * y_tt[p, (j,i)]
                      zc = zcp.tile([128, 8 * N_BIL * D], BF16, tag="zc")
                      z4 = zc[:].rearrange("p (t j i) -> p t j i",
                                           j=N_BIL, i=D)
                      for tt in range(8):
                          y = py.tile([128, N_BIL * D], F32, tag="y")
                          lhsT = me_sb[:, tt * 128:(tt + 1) * 128]
                          nc.tensor.matmul(y[:, 0:512], lhsT, t2_sb[:, 0:512],
                                           start=True, stop=True)
                          nc.tensor.matmul(y[:, 512:1024], lhsT,
                                           t2_sb[:, 512:1024],
                                           start=True, stop=True)
                          g8 = (c * 8 + tt) * N_BIL
                          if (c * 8 + tt) % 2 == 0:
                              for j in range(N_BIL):
                                  nc.scalar.activation(
                                      z4[:, tt:tt + 1, j:j + 1, :],
                                      y[:, j * D:(j + 1) * D],
                                      AF.Copy,
                                      scale=s_sbuf[:, g8 + j:g8 + j + 1])
                          else:
                              y3 = y[:].rearrange("p (j i) -> p j i", i=D)
                              s3 = s_sbuf[:, g8:g8 + N_BIL].rearrange(
                                  "p (j o) -> p j o", o=1).broadcast_to(
                                  [128, N_BIL, D])
                              nc.vector.tensor_mul(
                                  z4[:, tt:tt + 1, :, :], y3, s3)
                      # j-tree: 8 -> 4 (gpsimd), 4 -> 2, 2 -> 1 (DVE)
                      zt1 = trp.tile([128, 8 * 4 * D], BF16, tag="zt1")
                      t1v = zt1[:].rearrange("p (t j i) -> p t j i", j=4, i=D)
                      nc.gpsimd.tensor_add(t1v, z4[:, :, 0:4, :],
                                           z4[:, :, 4:8, :])
                      zt2 = trp.tile([128, 8 * 2 * D], BF16, tag="zt2")
                      t2v = zt2[:].rearrange("p (t j i) -> p t j i", j=2, i=D)
                      nc.vector.tensor_add(t2v, t1v[:, :, 0:2, :],
                                           t1v[:, :, 2:4, :])
                      of = trp.tile([128, 8 * D], F32, tag="of")
                      ofv = of[:].rearrange("p (t i) -> p t i", i=D)
                      nc.vector.tensor_add(ofv, t2v[:, :, 0:1, :],
                                           t2v[:, :, 1:2, :])
                      nc.sync.dma_start(out=ov[:, c * 8:(c + 1) * 8, :],
                                        in_=ofv)
            if 2 not in phases:
                ov = outd.ap().rearrange("(p x) c -> p (x c)", p=128)
                nc.sync.dma_start(out=ov[:, 0:NG * N_BIL], in_=s_sbuf[:])
    nc.finalize()
    return nc


# ----------------------------------------------------------------------------
# host-side sharding / unsharding
# ----------------------------------------------------------------------------

def make_cfg(kj, n_edges, ev=25_000, ep=26_624):
    n_cores = (n_edges + ev - 1) // ev
    owner = np.minimum(kj // ev, n_cores - 1)
    ng = ep // 128
    rg = np.zeros(ng, np.int64)
    for c in range(n_cores):
        loc = kj[owner == c] - c * ev
        cnt = np.bincount(loc, minlength=ev)
        s = np.zeros(ep, np.int64)
        s[:ev] = np.sort(cnt)[::-1]
        gmax = s.reshape(ng, 128).max(axis=1)
        rg = np.maximum(rg, (gmax + 1) // 2)
    return Cfg(ev, ep, tuple(int(r) for r in rg))


def prep_in_maps(cfg: Cfg, m_ji, nbr_list, angle_list, e_rbf, a_sbf, kj_idx,
                 W_nbr, b_nbr, W_e, W_a, final_w):
    del nbr_list, angle_list
    m_ji = np.asarray(m_ji, np.float32)
    e_rbf = np.asarray(e_rbf, np.float32)
    a_sbf = np.asarray(a_sbf, np.float32)
    kj = np.asarray(kj_idx).astype(np.int64)
    W_nbr = np.asarray(W_nbr, np.float32)
    b_nbr = np.asarray(b_nbr, np.float32)
    W_e = np.asarray(W_e, np.float32)
    W_a = np.asarray(W_a, np.float32)
    final_w = np.asarray(final_w, np.float32)

    n_edges = m_ji.shape[0]
    ev = cfg.e_valid
    ep = cfg.e_pad
    n_cores = (n_edges + ev - 1) // ev
    owner = np.minimum(kj // ev, n_cores - 1)

    wa2 = np.zeros((AT_P, N_BIL), np.float32)
    wa2[0:A_DIM] = W_a
    wa2[A_DIM:2 * A_DIM] = W_a
    t2 = np.ascontiguousarray(final_w.transpose(2, 1, 0).reshape(D, N_BIL * D))
    bn = np.ascontiguousarray(b_nbr.reshape(D, 1))

    in_maps = []
    perms = []
    for c in range(n_cores):
        sel = np.nonzero(owner == c)[0]
        loc = kj[sel] - c * ev
        cnt = np.bincount(loc, minlength=ev)
        edge_order = np.argsort(-cnt, kind="stable")     # slot -> local edge
        slot_of_edge = np.empty(ev, np.int64)
        slot_of_edge[edge_order] = np.arange(ev)
        ang_slot = slot_of_edge[loc]
        order = np.argsort(ang_slot, kind="stable")
        rows = sel[order]                 # a_sbf row per (slot-sorted) token
        cnt_slot = np.bincount(ang_slot, minlength=ep)
        starts = np.concatenate([[0], np.cumsum(cnt_slot)])

        at = np.zeros((AT_P, cfg.at_cols), np.float32)
        col = 0
        for g in range(cfg.n_groups):
            sl = np.arange(g * 128, (g + 1) * 128)
            csl = cnt_slot[sl]
            for p in range(cfg.rg[g]):
                for half, r in ((0, 2 * p), (1, 2 * p + 1)):
                    has = np.nonzero(csl > r)[0]
                    if len(has):
                        tok = starts[sl[has]] + r
                        at[half * A_DIM:(half + 1) * A_DIM,
                           col + has] = a_sbf[rows[tok]].T
                col += 128
        assert col == cfg.n_blocks * 128

        e0, e1 = c * ev, min((c + 1) * ev, n_edges)
        mjiT = np.zeros((D, ep), np.float32)
        mjiT[:, :e1 - e0] = m_ji[e0:e1][edge_order[:e1 - e0]].T
        erbfT = np.zeros((N_RBF, ep), np.float32)
        erbfT[:, :e1 - e0] = e_rbf[e0:e1][edge_order[:e1 - e0]].T

        bf = mybir.dt.np(BF16)
        im = {
            "a_t": at.astype(bf), "mji_t": mjiT.astype(bf),
            "erbf_t": erbfT.astype(bf), "w_nbr": W_nbr.astype(bf),
            "b_nbr": bn, "w_e": W_e.astype(bf), "w_a2": wa2.astype(bf),
            "t2": t2.astype(bf),
        }
        in_maps.append(im)
        perms.append(edge_order)
    return in_maps, perms


def gather_output(cfg: Cfg, results, perms, n_edges):
    ev = cfg.e_valid
    out = np.empty((n_edges, D), np.float32)
    for c, r in enumerate(results):
        e0, e1 = c * ev, min((c + 1) * ev, n_edges)
        out[e0 + perms[c][:e1 - e0]] = np.asarray(r["out"])[:e1 - e0]
    return out


_NC_CACHE = {}


def run_on_hw(inputs, cfg=None, trace=False, trace_cores=None, phases=(1, 2)):
    kj = np.asarray(inputs["kj_idx"]).astype(np.int64)
    if cfg is None:
        cfg = make_cfg(kj, inputs["m_ji"].shape[0])
    key = (cfg.key(), phases)
    if key not in _NC_CACHE:
        _NC_CACHE[key] = build_nc(cfg, phases=phases)
    nc = _NC_CACHE[key]
    in_maps, perms = prep_in_maps(cfg, **inputs)
    res = bass_utils.run_bass_kernel_spmd(
        nc, in_maps, core_ids=list(range(len(in_maps))),
        trace=trace, trace_cores=trace_cores)
    out = gather_output(cfg, res.results, perms, inputs["m_ji"].shape[0])
    return out, res


def kernel(**inputs) -> np.ndarray:
    out, _ = run_on_hw(inputs)
    return out


# revision 8
# speedup vs baseline: 6.5165x; 1.5764x over previous
"""Trainium2 Bass kernel for the DimeNet-style directed-message block.

Reference computation (W = n_angles, E = n_edges, D = 128, A = 49, J = 8):
    m_kj     = m_ji[kj_idx]                          # [W, D]
    transf_m = silu(m_kj @ W_nbr + b_nbr)            # [W, D]
    transf_e = e_rbf[kj_idx] @ W_e                   # [W, D]
    m_and_e  = transf_m * transf_e                   # [W, D]
    transf_a = a_sbf @ W_a                           # [W, J]
    out[w,i] = sum_{j,l} transf_a[w,j] m_and_e[w,l] final_w[i,j,l]
    final    = segment_sum(out, kj_idx, E)           # [E, D]

Algebraic refactor: the segment sum commutes through the bilinear form:
    me       = silu(m_ji @ W_nbr + b) * (e_rbf @ W_e)        # [E, D]
    S        = segment_sum(a_sbf @ W_a, kj_idx, E)           # [E, J]
    final[e] = sum_j S[e,j] * (me[e] @ final_w[:,j,:].T)     # [E, D]

S without scatter: edges are sharded contiguously (25000/core, angles
binned by owner core kj // 25000) and permuted within the core by
descending angle multiplicity.  Each 128-edge group g gets a static
rank-pair count rg[g] (cross-core max); the host packs the angles as
[98, 128] blocks (rank 2p in partitions 0:49, 2p+1 in 49:98), so

    S^T[:, group g] = sum_p [W_a; W_a]^T @ aT_block(g, p)    # [8, 128]

is a plain PSUM accumulation (feature-major S).  Descending sort makes the
rank profile a staircase: ~12% padding, no overflow level, no scatter.

The apply keeps everything feature-major.  S^T round-trips through DRAM and
is re-read with a partition-broadcast DMA (each SBUF partition reads the
same DRAM bytes), giving s_bc[l, (j,e)] = S[e,j] on all 128 partitions.
Then per chunk of 1024 edges:
    z_j  = me * s_bc_j                  # DVE bf16 2x, feature-major
    outT = sum_j final_w[:,j,:] @ z_j   # PSUM accumulation over j
and outT [D, E] is written bf16; the host transposes/casts/unpermutes.
"""

import numpy as np

import concourse.bass as bass
import concourse.mybir as mybir
import concourse.tile as tile
from concourse import bacc, bass_utils

F32 = mybir.dt.float32
BF16 = mybir.dt.bfloat16
AF = mybir.ActivationFunctionType
OP = mybir.AluOpType

D = 128
A_DIM = 49
N_RBF = 6
N_BIL = 8
N_CORES = 8
AT_P = 2 * A_DIM          # 98 partitions: even rank 0:49, odd rank 49:98
AT_TILE = 4096            # aT stream tile width (cols); 32 blocks per tile


class Cfg:
    def __init__(self, e_valid, e_pad, rg):
        self.e_valid = e_valid
        self.e_pad = e_pad
        self.rg = tuple(int(r) for r in rg)      # rank-pairs per 128-edge group
        assert e_pad % 1024 == 0
        self.n_groups = e_pad // 128
        assert len(self.rg) == self.n_groups
        self.n_blocks = sum(self.rg)
        self.at_cols = ((self.n_blocks * 128 + AT_TILE - 1) // AT_TILE) * AT_TILE
        self.n_chunks = e_pad // 1024

    def key(self):
        return (self.e_valid, self.e_pad, self.rg)


def build_nc(cfg: Cfg):
    nc = bacc.Bacc(None)
    EP = cfg.e_pad
    NG = cfg.n_groups
    NC = cfg.n_chunks

    aT = nc.dram_tensor("a_t", [AT_P, cfg.at_cols], BF16, kind="ExternalInput")
    mjiT = nc.dram_tensor("mji_t", [D, EP], BF16, kind="ExternalInput")
    erbf = nc.dram_tensor("erbf_t", [N_RBF, EP], BF16, kind="ExternalInput")
    wnbr = nc.dram_tensor("w_nbr", [D, D], BF16, kind="ExternalInput")
    bnbr = nc.dram_tensor("b_nbr", [D, 1], F32, kind="ExternalInput")
    wes = nc.dram_tensor("w_e", [N_RBF, D], BF16, kind="ExternalInput")
    wa2 = nc.dram_tensor("w_a2", [AT_P, N_BIL], BF16, kind="ExternalInput")
    t2 = nc.dram_tensor("t2", [D, N_BIL * D], BF16, kind="ExternalInput")
    outd = nc.dram_tensor("out", [D, EP], BF16, kind="ExternalOutput")
    sTd = nc.dram_tensor("s_t", [NC, N_BIL, 1024], BF16)   # chunk-major S^T

    with tile.TileContext(nc) as tc:
        with tc.tile_pool(name="const", bufs=1) as cp:
            wa_sb = cp.tile([AT_P, N_BIL], BF16)
            nc.sync.dma_start(out=wa_sb[:], in_=wa2[:])
            sT_sb = cp.tile([N_BIL, EP], BF16)
            nc.vector.memset(sT_sb[:], 0.0)

            # ====== phase A: S^T via per-group PSUM rank accumulation ======
            with tc.tile_pool(name="pa", bufs=3) as pa, \
                 tc.tile_pool(name="pss", bufs=2, space="PSUM") as pss:
                at_tiles = {}

                def at_block(b):
                    tk = b // (AT_TILE // 128)
                    if tk not in at_tiles:
                        t = pa.tile([AT_P, AT_TILE], BF16, tag="at")
                        nc.sync.dma_start(
                            out=t[:], in_=aT[:, tk * AT_TILE:(tk + 1) * AT_TILE])
                        at_tiles.clear()
                        at_tiles[tk] = t
                    off = (b % (AT_TILE // 128)) * 128
                    return at_tiles[tk][:, off:off + 128]

                blk = 0
                for g0 in range(0, NG, 4):
                    g1 = g0 + 4
                    nz = sum(1 for g in range(g0, g1) if cfg.rg[g] > 0)
                    if nz == 0:
                        continue
                    ps = pss.tile([N_BIL, 512], F32, tag="ps")
                    for g in range(g0, g1):
                        R = cfg.rg[g]
                        if R == 0:
                            continue
                        sl = (g - g0) * 128
                        for p in range(R):
                            nc.tensor.matmul(
                                ps[:, sl:sl + 128], wa_sb[:], at_block(blk),
                                start=(p == 0), stop=(p == R - 1))
                            blk += 1
                    nc.scalar.activation(
                        sT_sb[:, g0 * 128:(g0 + nz) * 128], ps[:, :nz * 128],
                        AF.Copy)
                # spill S^T chunk-major so phase B can partition-broadcast it
                for c in range(NC):
                    nc.sync.dma_start(
                        out=sTd.ap()[c:c + 1, :, :].squeeze(0),
                        in_=sT_sb[:, c * 1024:(c + 1) * 1024])

            # ============ phase B: edge transform + S apply ================
            wn_sb = cp.tile([D, D], BF16)
            nc.sync.dma_start(out=wn_sb[:], in_=wnbr[:])
            b_sb = cp.tile([D, 1], F32)
            nc.sync.dma_start(out=b_sb[:], in_=bnbr[:])
            we_sb = cp.tile([N_RBF, D], BF16)
            nc.sync.dma_start(out=we_sb[:], in_=wes[:])
            t2_sb = cp.tile([D, N_BIL * D], BF16)
            nc.sync.dma_start(out=t2_sb[:], in_=t2[:])

            with tc.tile_pool(name="pb", bufs=2) as pb, \
                 tc.tile_pool(name="zp", bufs=2) as zp, \
                 tc.tile_pool(name="ofp", bufs=2) as ofp, \
                 tc.tile_pool(name="psmm", bufs=2, space="PSUM") as pmm, \
                 tc.tile_pool(name="psy", bufs=2, space="PSUM") as py:
                for c in range(NC):
                    s_bc = pb.tile([128, N_BIL * 1024], BF16, tag="sbc")
                    nc.sync.dma_start(
                        out=s_bc[:],
                        in_=sTd.ap()[c:c + 1, :, :].broadcast_to(
                            [128, N_BIL, 1024]))
                    er_sb = pb.tile([N_RBF, 1024], BF16, tag="er")
                    nc.sync.dma_start(out=er_sb[:],
                                      in_=erbf[:, c * 1024:(c + 1) * 1024])
                    te_ps = pmm.tile([128, 1024], F32, tag="mm")
                    for n in range(2):
                        nc.tensor.matmul(
                            te_ps[:, n * 512:(n + 1) * 512],
                            we_sb[:], er_sb[:, n * 512:(n + 1) * 512],
                            start=True, stop=True)
                    te_sb = pb.tile([128, 1024], BF16, tag="te")
                    nc.scalar.activation(te_sb[:], te_ps[:], AF.Copy)
                    mj = pb.tile([128, 1024], BF16, tag="mj")
                    nc.sync.dma_start(out=mj[:],
                                      in_=mjiT[:, c * 1024:(c + 1) * 1024])
                    tm_ps = pmm.tile([128, 1024], F32, tag="mm")
                    for n in range(2):
                        nc.tensor.matmul(
                            tm_ps[:, n * 512:(n + 1) * 512],
                            wn_sb[:], mj[:, n * 512:(n + 1) * 512],
                            start=True, stop=True)
                    tm_sb = pb.tile([128, 1024], BF16, tag="tm")
                    nc.scalar.activation(tm_sb[:], tm_ps[:], AF.Silu,
                                         bias=b_sb[:, 0:1])
                    me_sb = pb.tile([128, 1024], BF16, tag="me")
                    nc.vector.tensor_mul(me_sb[:], tm_sb[:], te_sb[:])

                    z = zp.tile([128, N_BIL * 1024], BF16, tag="z")
                    for j in range(N_BIL):
                        nc.vector.tensor_mul(
                            z[:, j * 1024:(j + 1) * 1024], me_sb[:],
                            s_bc[:, j * 1024:(j + 1) * 1024])
                    ot = py.tile([128, 1024], F32, tag="ot")
                    for h in range(2):
                        for j in range(N_BIL):
                            nc.tensor.matmul(
                                ot[:, h * 512:(h + 1) * 512],
                                t2_sb[:, j * 128:(j + 1) * 128],
                                z[:, j * 1024 + h * 512:j * 1024 + (h + 1) * 512],
                                start=(j == 0), stop=(j == N_BIL - 1))
                    of = ofp.tile([128, 1024], BF16, tag="of")
                    nc.scalar.activation(of[:], ot[:], AF.Copy)
                    nc.sync.dma_start(
                        out=outd[:, c * 1024:(c + 1) * 1024], in_=of[:])
    nc.finalize()
    return nc


# ----------------------------------------------------------------------------
# host-side sharding / unsharding
# ----------------------------------------------------------------------------

def make_cfg(kj, n_edges, ev=25_000, ep=26_624):
    n_cores = (n_edges + ev - 1) // ev
    owner = np.minimum(kj // ev, n_cores - 1)
    ng = ep // 128
    rg = np.zeros(ng, np.int64)
    for c in range(n_cores):
        loc = kj[owner == c] - c * ev
        cnt = np.bincount(loc, minlength=ev)
        s = np.zeros(ep, np.int64)
        s[:ev] = np.sort(cnt)[::-1]
        gmax = s.reshape(ng, 128).max(axis=1)
        rg = np.maximum(rg, (gmax + 1) // 2)
    return Cfg(ev, ep, tuple(int(r) for r in rg))


def prep_in_maps(cfg: Cfg, m_ji, nbr_list, angle_list, e_rbf, a_sbf, kj_idx,
                 W_nbr, b_nbr, W_e, W_a, final_w):
    del nbr_list, angle_list
    m_ji = np.asarray(m_ji, np.float32)
    e_rbf = np.asarray(e_rbf, np.float32)
    a_sbf = np.asarray(a_sbf, np.float32)
    kj = np.asarray(kj_idx).astype(np.int64)
    W_nbr = np.asarray(W_nbr, np.float32)
    b_nbr = np.asarray(b_nbr, np.float32)
    W_e = np.asarray(W_e, np.float32)
    W_a = np.asarray(W_a, np.float32)
    final_w = np.asarray(final_w, np.float32)

    n_edges = m_ji.shape[0]
    ev = cfg.e_valid
    ep = cfg.e_pad
    n_cores = (n_edges + ev - 1) // ev
    owner = np.minimum(kj // ev, n_cores - 1)

    wa2 = np.zeros((AT_P, N_BIL), np.float32)
    wa2[0:A_DIM] = W_a
    wa2[A_DIM:2 * A_DIM] = W_a
    t2 = np.ascontiguousarray(final_w.transpose(2, 1, 0).reshape(D, N_BIL * D))
    bn = np.ascontiguousarray(b_nbr.reshape(D, 1))

    in_maps = []
    perms = []
    for c in range(n_cores):
        sel = np.nonzero(owner == c)[0]
        loc = kj[sel] - c * ev
        cnt = np.bincount(loc, minlength=ev)
        edge_order = np.argsort(-cnt, kind="stable")     # slot -> local edge
        slot_of_edge = np.empty(ev, np.int64)
        slot_of_edge[edge_order] = np.arange(ev)
        ang_slot = slot_of_edge[loc]
        order = np.argsort(ang_slot, kind="stable")
        rows = sel[order]                 # a_sbf row per (slot-sorted) token
        cnt_slot = np.bincount(ang_slot, minlength=ep)
        starts = np.concatenate([[0], np.cumsum(cnt_slot)])

        at = np.zeros((AT_P, cfg.at_cols), np.float32)
        col = 0
        for g in range(cfg.n_groups):
            sl = np.arange(g * 128, (g + 1) * 128)
            csl = cnt_slot[sl]
            for p in range(cfg.rg[g]):
                for half, r in ((0, 2 * p), (1, 2 * p + 1)):
                    has = np.nonzero(csl > r)[0]
                    if len(has):
                        tok = starts[sl[has]] + r
                        at[half * A_DIM:(half + 1) * A_DIM,
                           col + has] = a_sbf[rows[tok]].T
                col += 128
        assert col == cfg.n_blocks * 128

        e0, e1 = c * ev, min((c + 1) * ev, n_edges)
        mjiT = np.zeros((D, ep), np.float32)
        mjiT[:, :e1 - e0] = m_ji[e0:e1][edge_order[:e1 - e0]].T
        erbfT = np.zeros((N_RBF, ep), np.float32)
        erbfT[:, :e1 - e0] = e_rbf[e0:e1][edge_order[:e1 - e0]].T

        bf = mybir.dt.np(BF16)
        im = {
            "a_t": at.astype(bf), "mji_t": mjiT.astype(bf),
            "erbf_t": erbfT.astype(bf), "w_nbr": W_nbr.astype(bf),
            "b_nbr": bn, "w_e": W_e.astype(bf), "w_a2": wa2.astype(bf),
            "t2": t2.astype(bf),
        }
        in_maps.append(im)
        perms.append(edge_order)
    return in_maps, perms


def gather_output(cfg: Cfg, results, perms, n_edges):
    ev = cfg.e_valid
    out = np.empty((n_edges, D), np.float32)
    for c, r in enumerate(results):
        e0, e1 = c * ev, min((c + 1) * ev, n_edges)
        dev = np.asarray(r["out"]).astype(np.float32)       # [D, EP]
        out[e0 + perms[c][:e1 - e0]] = dev[:, :e1 - e0].T
    return out


_NC_CACHE = {}


def run_on_hw(inputs, cfg=None, trace=False, trace_cores=None):
    kj = np.asarray(inputs["kj_idx"]).astype(np.int64)
    if cfg is None:
        cfg = make_cfg(kj, inputs["m_ji"].shape[0])
    key = cfg.key()
    if key not in _NC_CACHE:
        _NC_CACHE[key] = build_nc(cfg)
    nc = _NC_CACHE[key]
    in_maps, perms = prep_in_maps(cfg, **inputs)
    res = bass_utils.run_bass_kernel_spmd(
        nc, in_maps, core_ids=list(range(len(in_maps))),
        trace=trace, trace_cores=trace_cores)
    out = gather_output(cfg, res.results, perms, inputs["m_ji"].shape[0])
    return out, res


def kernel(**inputs) -> np.ndarray:
    out, _ = run_on_hw(inputs)
    return out


# revision 10
# speedup vs baseline: 6.9928x; 1.0731x over previous
"""Trainium2 Bass kernel for the DimeNet-style directed-message block.

Reference computation (W = n_angles, E = n_edges, D = 128, A = 49, J = 8):
    m_kj     = m_ji[kj_idx]                          # [W, D]
    transf_m = silu(m_kj @ W_nbr + b_nbr)            # [W, D]
    transf_e = e_rbf[kj_idx] @ W_e                   # [W, D]
    m_and_e  = transf_m * transf_e                   # [W, D]
    transf_a = a_sbf @ W_a                           # [W, J]
    out[w,i] = sum_{j,l} transf_a[w,j] m_and_e[w,l] final_w[i,j,l]
    final    = segment_sum(out, kj_idx, E)           # [E, D]

Algebraic refactor: the segment sum commutes through the bilinear form:
    me       = silu(m_ji @ W_nbr + b) * (e_rbf @ W_e)        # [E, D]
    S        = segment_sum(a_sbf @ W_a, kj_idx, E)           # [E, J]
    final[e] = sum_j S[e,j] * (me[e] @ final_w[:,j,:].T)     # [E, D]

S without scatter: edges are sharded contiguously (25000/core, angles
binned by owner core kj // 25000) and permuted within the core by
descending angle multiplicity.  Each 128-edge group g gets a static
rank-pair count rg[g] (cross-core max); the host packs the angles as
[98, 128] blocks (rank 2p in partitions 0:49, 2p+1 in 49:98), so

    S^T[:, group g] = sum_p [W_a; W_a]^T @ aT_block(g, p)    # [8, 128]

is a plain PSUM accumulation (feature-major S).  Descending sort makes the
rank profile a staircase: ~12% padding, no overflow level, no scatter.

The apply keeps everything feature-major.  S^T round-trips through DRAM and
is re-read with a partition-broadcast DMA (each SBUF partition reads the
same DRAM bytes), giving s_bc[l, (j,e)] = S[e,j] on all 128 partitions.
Then per chunk of 1024 edges:
    z_j  = me * s_bc_j                  # DVE bf16 2x, feature-major
    outT = sum_j final_w[:,j,:] @ z_j   # PSUM accumulation over j
and outT [D, E] is written bf16; the host transposes/casts/unpermutes.
"""

import numpy as np

import concourse.bass as bass
import concourse.mybir as mybir
import concourse.tile as tile
from concourse import bacc, bass_utils

F32 = mybir.dt.float32
BF16 = mybir.dt.bfloat16
AF = mybir.ActivationFunctionType
OP = mybir.AluOpType

D = 128
A_DIM = 49
N_RBF = 6
N_BIL = 8
N_CORES = 8
AT_P = 2 * A_DIM          # 98 partitions: even rank 0:49, odd rank 49:98
AT_TILE = 4096            # aT stream tile width (cols); 32 blocks per tile


class Cfg:
    def __init__(self, e_valid, e_pad, rg):
        self.e_valid = e_valid
        self.e_pad = e_pad
        self.rg = tuple(int(r) for r in rg)      # rank-pairs per 128-edge group
        assert e_pad % 1024 == 0
        self.n_groups = e_pad // 128
        assert len(self.rg) == self.n_groups
        self.n_blocks = sum(self.rg)
        self.at_cols = ((self.n_blocks * 128 + AT_TILE - 1) // AT_TILE) * AT_TILE
        self.n_chunks = e_pad // 1024

    def key(self):
        return (self.e_valid, self.e_pad, self.rg)


def build_nc(cfg: Cfg):
    nc = bacc.Bacc(None)
    EP = cfg.e_pad
    NG = cfg.n_groups
    NC = cfg.n_chunks

    aT = nc.dram_tensor("a_t", [AT_P, cfg.at_cols], BF16, kind="ExternalInput")
    mjiT = nc.dram_tensor("mji_t", [D, EP], BF16, kind="ExternalInput")
    erbf = nc.dram_tensor("erbf_t", [N_RBF, EP], BF16, kind="ExternalInput")
    wnbr = nc.dram_tensor("w_nbr", [D, D], BF16, kind="ExternalInput")
    bnbr = nc.dram_tensor("b_nbr", [D, 1], F32, kind="ExternalInput")
    wes = nc.dram_tensor("w_e", [N_RBF, D], BF16, kind="ExternalInput")
    wa2 = nc.dram_tensor("w_a2", [AT_P, N_BIL], BF16, kind="ExternalInput")
    t2 = nc.dram_tensor("t2", [D, N_BIL * D], BF16, kind="ExternalInput")
    outd = nc.dram_tensor("out", [D, EP], BF16, kind="ExternalOutput")
    # chunk-major S^T spill, one tensor per chunk so phase B pipelines with A
    sTd = [nc.dram_tensor(f"s_t{c}", [N_BIL, 1024], BF16) for c in range(NC)]

    with tile.TileContext(nc) as tc:
        with tc.tile_pool(name="const", bufs=1) as cp:
            wa_sb = cp.tile([AT_P, N_BIL], BF16)
            nc.sync.dma_start(out=wa_sb[:], in_=wa2[:])

            # ====== phase A: S^T via per-group PSUM rank accumulation ======
            with tc.tile_pool(name="pa", bufs=3) as pa, \
                 tc.tile_pool(name="stp", bufs=3) as stp, \
                 tc.tile_pool(name="pss", bufs=2, space="PSUM") as pss:
                at_tiles = {}

                def at_block(b):
                    tk = b // (AT_TILE // 128)
                    if tk not in at_tiles:
                        t = pa.tile([AT_P, AT_TILE], BF16, tag="at")
                        nc.sync.dma_start(
                            out=t[:], in_=aT[:, tk * AT_TILE:(tk + 1) * AT_TILE])
                        at_tiles.clear()
                        at_tiles[tk] = t
                    off = (b % (AT_TILE // 128)) * 128
                    return at_tiles[tk][:, off:off + 128]

                blk = 0
                for c in range(NC):
                    st = stp.tile([N_BIL, 1024], BF16, tag="st")
                    nzc = sum(1 for g in range(c * 8, c * 8 + 8)
                              if cfg.rg[g] > 0)
                    if nzc < 8:
                        nc.vector.memset(st[:], 0.0)
                    for half in range(2):
                        g0 = c * 8 + half * 4
                        nz = sum(1 for g in range(g0, g0 + 4)
                                 if cfg.rg[g] > 0)
                        if nz == 0:
                            continue
                        ps = pss.tile([N_BIL, 512], F32, tag="ps")
                        for g in range(g0, g0 + 4):
                            R = cfg.rg[g]
                            if R == 0:
                                continue
                            sl = (g - g0) * 128
                            for p in range(R):
                                nc.tensor.matmul(
                                    ps[:, sl:sl + 128], wa_sb[:],
                                    at_block(blk),
                                    start=(p == 0), stop=(p == R - 1))
                                blk += 1
                        nc.scalar.activation(
                            st[:, half * 512:half * 512 + nz * 128],
                            ps[:, :nz * 128], AF.Copy)
                    nc.sync.dma_start(out=sTd[c].ap(), in_=st[:])

            # ============ phase B: edge transform + S apply ================
            wn_sb = cp.tile([D, D], BF16)
            nc.sync.dma_start(out=wn_sb[:], in_=wnbr[:])
            b_sb = cp.tile([D, 1], F32)
            nc.sync.dma_start(out=b_sb[:], in_=bnbr[:])
            we_sb = cp.tile([N_RBF, D], BF16)
            nc.sync.dma_start(out=we_sb[:], in_=wes[:])
            t2_sb = cp.tile([D, N_BIL * D], BF16)
            nc.sync.dma_start(out=t2_sb[:], in_=t2[:])

            with tc.tile_pool(name="pb", bufs=2) as pb, \
                 tc.tile_pool(name="mjp", bufs=2) as mjp, \
                 tc.tile_pool(name="zp", bufs=2) as zp, \
                 tc.tile_pool(name="ofp", bufs=2) as ofp, \
                 tc.tile_pool(name="psmm", bufs=2, space="PSUM") as pmm, \
                 tc.tile_pool(name="psy", bufs=1, space="PSUM") as py:
                mj2 = of2 = None
                for c in range(NC):
                    s_bc = pb.tile([128, N_BIL * 1024], BF16, tag="sbc")
                    nc.sync.dma_start(
                        out=s_bc[:],
                        in_=sTd[c].ap().unsqueeze(0).broadcast_to(
                            [128, N_BIL, 1024]))
                    er_sb = pb.tile([N_RBF, 1024], BF16, tag="er")
                    nc.sync.dma_start(out=er_sb[:],
                                      in_=erbf[:, c * 1024:(c + 1) * 1024])
                    te_ps = pmm.tile([128, 1024], F32, tag="mm")
                    for n in range(2):
                        nc.tensor.matmul(
                            te_ps[:, n * 512:(n + 1) * 512],
                            we_sb[:], er_sb[:, n * 512:(n + 1) * 512],
                            start=True, stop=True)
                    te_sb = pb.tile([128, 1024], BF16, tag="te")
                    nc.scalar.activation(te_sb[:], te_ps[:], AF.Copy)
                    if c % 2 == 0:
                        mj2 = mjp.tile([128, 2048], BF16, tag="mj")
                        cend = min(c + 2, NC)
                        nc.sync.dma_start(
                            out=mj2[:, :(cend - c) * 1024],
                            in_=mjiT[:, c * 1024:cend * 1024])
                    mj = mj2[:, (c % 2) * 1024:(c % 2 + 1) * 1024]
                    tm_ps = pmm.tile([128, 1024], F32, tag="mm")
                    for n in range(2):
                        nc.tensor.matmul(
                            tm_ps[:, n * 512:(n + 1) * 512],
                            wn_sb[:], mj[:, n * 512:(n + 1) * 512],
                            start=True, stop=True)
                    tm_sb = pb.tile([128, 1024], BF16, tag="tm")
                    nc.scalar.activation(tm_sb[:], tm_ps[:], AF.Silu,
                                         bias=b_sb[:, 0:1])
                    me_sb = pb.tile([128, 1024], BF16, tag="me")
                    nc.vector.tensor_mul(me_sb[:], tm_sb[:], te_sb[:])

                    z = zp.tile([128, N_BIL * 1024], BF16, tag="z")
                    for j in range(N_BIL):
                        nc.vector.tensor_mul(
                            z[:, j * 1024:(j + 1) * 1024], me_sb[:],
                            s_bc[:, j * 1024:(j + 1) * 1024])
                    ot = py.tile([128, 1024], F32, tag="ot")
                    for h in range(2):
                        for j in range(N_BIL):
                            nc.tensor.matmul(
                                ot[:, h * 512:(h + 1) * 512],
                                t2_sb[:, j * 128:(j + 1) * 128],
                                z[:, j * 1024 + h * 512:j * 1024 + (h + 1) * 512],
                                start=(j == 0), stop=(j == N_BIL - 1))
                    if c % 2 == 0:
                        of2 = ofp.tile([128, 2048], BF16, tag="of")
                    nc.scalar.activation(
                        of2[:, (c % 2) * 1024:(c % 2 + 1) * 1024], ot[:],
                        AF.Copy)
                    if c % 2 == 1 or c == NC - 1:
                        c0 = c - (c % 2)
                        nc.sync.dma_start(
                            out=outd[:, c0 * 1024:(c + 1) * 1024],
                            in_=of2[:, :(c + 1 - c0) * 1024])
    nc.finalize()
    return nc


# ----------------------------------------------------------------------------
# host-side sharding / unsharding
# ----------------------------------------------------------------------------

def make_cfg(kj, n_edges, ev=25_000, ep=26_624):
    n_cores = (n_edges + ev - 1) // ev
    owner = np.minimum(kj // ev, n_cores - 1)
    ng = ep // 128
    rg = np.zeros(ng, np.int64)
    for c in range(n_cores):
        loc = kj[owner == c] - c * ev
        cnt = np.bincount(loc, minlength=ev)
        s = np.zeros(ep, np.int64)
        s[:ev] = np.sort(cnt)[::-1]
        gmax = s.reshape(ng, 128).max(axis=1)
        rg = np.maximum(rg, (gmax + 1) // 2)
    return Cfg(ev, ep, tuple(int(r) for r in rg))


def prep_in_maps(cfg: Cfg, m_ji, nbr_list, angle_list, e_rbf, a_sbf, kj_idx,
                 W_nbr, b_nbr, W_e, W_a, final_w):
    del nbr_list, angle_list
    m_ji = np.asarray(m_ji, np.float32)
    e_rbf = np.asarray(e_rbf, np.float32)
    a_sbf = np.asarray(a_sbf, np.float32)
    kj = np.asarray(kj_idx).astype(np.int64)
    W_nbr = np.asarray(W_nbr, np.float32)
    b_nbr = np.asarray(b_nbr, np.float32)
    W_e = np.asarray(W_e, np.float32)
    W_a = np.asarray(W_a, np.float32)
    final_w = np.asarray(final_w, np.float32)

    n_edges = m_ji.shape[0]
    ev = cfg.e_valid
    ep = cfg.e_pad
    n_cores = (n_edges + ev - 1) // ev
    owner = np.minimum(kj // ev, n_cores - 1)

    wa2 = np.zeros((AT_P, N_BIL), np.float32)
    wa2[0:A_DIM] = W_a
    wa2[A_DIM:2 * A_DIM] = W_a
    t2 = np.ascontiguousarray(final_w.transpose(2, 1, 0).reshape(D, N_BIL * D))
    bn = np.ascontiguousarray(b_nbr.reshape(D, 1))

    in_maps = []
    perms = []
    for c in range(n_cores):
        sel = np.nonzero(owner == c)[0]
        loc = kj[sel] - c * ev
        cnt = np.bincount(loc, minlength=ev)
        edge_order = np.argsort(-cnt, kind="stable")     # slot -> local edge
        slot_of_edge = np.empty(ev, np.int64)
        slot_of_edge[edge_order] = np.arange(ev)
        ang_slot = slot_of_edge[loc]
        order = np.argsort(ang_slot, kind="stable")
        rows = sel[order]                 # a_sbf row per (slot-sorted) token
        cnt_slot = np.bincount(ang_slot, minlength=ep)
        starts = np.concatenate([[0], np.cumsum(cnt_slot)])

        at = np.zeros((AT_P, cfg.at_cols), np.float32)
        col = 0
        for g in range(cfg.n_groups):
            sl = np.arange(g * 128, (g + 1) * 128)
            csl = cnt_slot[sl]
            for p in range(cfg.rg[g]):
                for half, r in ((0, 2 * p), (1, 2 * p + 1)):
                    has = np.nonzero(csl > r)[0]
                    if len(has):
                        tok = starts[sl[has]] + r
                        at[half * A_DIM:(half + 1) * A_DIM,
                           col + has] = a_sbf[rows[tok]].T
                col += 128
        assert col == cfg.n_blocks * 128

        e0, e1 = c * ev, min((c + 1) * ev, n_edges)
        mjiT = np.zeros((D, ep), np.float32)
        mjiT[:, :e1 - e0] = m_ji[e0:e1][edge_order[:e1 - e0]].T
        erbfT = np.zeros((N_RBF, ep), np.float32)
        erbfT[:, :e1 - e0] = e_rbf[e0:e1][edge_order[:e1 - e0]].T

        bf = mybir.dt.np(BF16)
        im = {
            "a_t": at.astype(bf), "mji_t": mjiT.astype(bf),
            "erbf_t": erbfT.astype(bf), "w_nbr": W_nbr.astype(bf),
            "b_nbr": bn, "w_e": W_e.astype(bf), "w_a2": wa2.astype(bf),
            "t2": t2.astype(bf),
        }
        in_maps.append(im)
        perms.append(edge_order)
    return in_maps, perms


def gather_output(cfg: Cfg, results, perms, n_edges):
    ev = cfg.e_valid
    out = np.empty((n_edges, D), np.float32)
    for c, r in enumerate(results):
        e0, e1 = c * ev, min((c + 1) * ev, n_edges)
        dev = np.asarray(r["out"]).astype(np.float32)       # [D, EP]
        out[e0 + perms[c][:e1 - e0]] = dev[:, :e1 - e0].T
    return out


_NC_CACHE = {}


def run_on_hw(inputs, cfg=None, trace=False, trace_cores=None):
    kj = np.asarray(inputs["kj_idx"]).astype(np.int64)
    if cfg is None:
        cfg = make_cfg(kj, inputs["m_ji"].shape[0])
    key = cfg.key()
    if key not in _NC_CACHE:
        _NC_CACHE[key] = build_nc(cfg)
    nc = _NC_CACHE[key]
    in_maps, perms = prep_in_maps(cfg, **inputs)
    res = bass_utils.run_bass_kernel_spmd(
        nc, in_maps, core_ids=list(range(len(in_maps))),
        trace=trace, trace_cores=trace_cores)
    out = gather_output(cfg, res.results, perms, inputs["m_ji"].shape[0])
    return out, res


def kernel(**inputs) -> np.ndarray:
    out, _ = run_on_hw(inputs)
    return out


# revision 11
# speedup vs baseline: 8.2745x; 1.1833x over previous
"""Trainium2 Bass kernel for the DimeNet-style directed-message block.

Reference computation (W = n_angles, E = n_edges, D = 128, A = 49, J = 8):
    m_kj     = m_ji[kj_idx]                          # [W, D]
    transf_m = silu(m_kj @ W_nbr + b_nbr)            # [W, D]
    transf_e = e_rbf[kj_idx] @ W_e                   # [W, D]
    m_and_e  = transf_m * transf_e                   # [W, D]
    transf_a = a_sbf @ W_a                           # [W, J]
    out[w,i] = sum_{j,l} transf_a[w,j] m_and_e[w,l] final_w[i,j,l]
    final    = segment_sum(out, kj_idx, E)           # [E, D]

Algebraic refactor: the segment sum commutes through the bilinear form:
    me       = silu(m_ji @ W_nbr + b) * (e_rbf @ W_e)        # [E, D]
    S        = segment_sum(a_sbf @ W_a, kj_idx, E)           # [E, J]
    final[e] = sum_j S[e,j] * (me[e] @ final_w[:,j,:].T)     # [E, D]

S without scatter: edges are sharded contiguously (25000/core, angles
binned by owner core kj // 25000) and permuted within the core by
descending angle multiplicity.  Each 128-edge group g gets a static
rank-pair count rg[g] (cross-core max); the host packs the angles as
[98, 128] blocks (rank 2p in partitions 0:49, 2p+1 in 49:98), so

    S^T[:, group g] = sum_p [W_a; W_a]^T @ aT_block(g, p)    # [8, 128]

is a plain PSUM accumulation (feature-major S).  Descending sort makes the
rank profile a staircase: ~12% padding, no overflow level, no scatter.

The apply keeps everything feature-major.  S^T round-trips through DRAM and
is re-read with a partition-broadcast DMA (each SBUF partition reads the
same DRAM bytes), giving s_bc[l, (j,e)] = S[e,j] on all 128 partitions.
Then per chunk of 1024 edges:
    z_j  = me * s_bc_j                  # DVE bf16 2x, feature-major
    outT = sum_j final_w[:,j,:] @ z_j   # PSUM accumulation over j
and outT [D, E] is written bf16; the host transposes/casts/unpermutes.
"""

import numpy as np

import concourse.bass as bass
import concourse.mybir as mybir
import concourse.tile as tile
from concourse import bacc, bass_utils

F32 = mybir.dt.float32
BF16 = mybir.dt.bfloat16
AF = mybir.ActivationFunctionType
OP = mybir.AluOpType

D = 128
A_DIM = 49
N_RBF = 6
N_BIL = 8
N_CORES = 8
AT_P = 2 * A_DIM          # 98 partitions: even rank 0:49, odd rank 49:98
AT_TILE = 4096            # aT stream tile width (cols); 32 blocks per tile


class Cfg:
    def __init__(self, e_valid, e_pad, rg):
        self.e_valid = e_valid
        self.e_pad = e_pad
        self.rg = tuple(int(r) for r in rg)      # rank-pairs per 128-edge group
        assert e_pad % 1024 == 0
        self.n_groups = e_pad // 128
        assert len(self.rg) == self.n_groups
        self.n_blocks = sum(self.rg)
        self.at_cols = ((self.n_blocks * 128 + AT_TILE - 1) // AT_TILE) * AT_TILE
        self.n_chunks = e_pad // 1024

    def key(self):
        return (self.e_valid, self.e_pad, self.rg)


def build_nc(cfg: Cfg):
    nc = bacc.Bacc(None)
    EP = cfg.e_pad
    NG = cfg.n_groups
    NC = cfg.n_chunks

    aT = nc.dram_tensor("a_t", [AT_P, cfg.at_cols], BF16, kind="ExternalInput")
    mjiT = nc.dram_tensor("mji_t", [D, EP], BF16, kind="ExternalInput")
    erbf = nc.dram_tensor("erbf_t", [N_RBF, EP], BF16, kind="ExternalInput")
    wnbr = nc.dram_tensor("w_nbr", [D, D], BF16, kind="ExternalInput")
    bnbr = nc.dram_tensor("b_nbr", [D, 1], F32, kind="ExternalInput")
    wes = nc.dram_tensor("w_e", [N_RBF, D], BF16, kind="ExternalInput")
    wa2 = nc.dram_tensor("w_a2", [AT_P, N_BIL], BF16, kind="ExternalInput")
    t2 = nc.dram_tensor("t2", [D, N_BIL * D], BF16, kind="ExternalInput")
    outd = nc.dram_tensor("out", [D, EP], BF16, kind="ExternalOutput")
    # chunk-major S^T spill, one tensor per chunk so phase B pipelines with A
    sTd = [nc.dram_tensor(f"s_t{c}", [N_BIL, 1024], BF16) for c in range(NC)]

    with tile.TileContext(nc) as tc:
        with tc.tile_pool(name="const", bufs=1) as cp:
            wa_sb = cp.tile([AT_P, N_BIL], BF16)
            nc.sync.dma_start(out=wa_sb[:], in_=wa2[:])

            # ====== phase A: S^T via per-group PSUM rank accumulation ======
            with tc.tile_pool(name="pa", bufs=4) as pa, \
                 tc.tile_pool(name="stp", bufs=3) as stp, \
                 tc.tile_pool(name="pss", bufs=2, space="PSUM") as pss:
                at_tiles = {}

                def at_block(b):
                    tk = b // (AT_TILE // 128)
                    if tk not in at_tiles:
                        t = pa.tile([AT_P, AT_TILE], BF16, tag="at")
                        nc.sync.dma_start(
                            out=t[:], in_=aT[:, tk * AT_TILE:(tk + 1) * AT_TILE])
                        at_tiles.clear()
                        at_tiles[tk] = t
                    off = (b % (AT_TILE // 128)) * 128
                    return at_tiles[tk][:, off:off + 128]

                blk = 0
                for c in range(NC):
                    st = stp.tile([N_BIL, 1024], BF16, tag="st")
                    nzc = sum(1 for g in range(c * 8, c * 8 + 8)
                              if cfg.rg[g] > 0)
                    if nzc < 8:
                        nc.vector.memset(st[:], 0.0)
                    for half in range(2):
                        g0 = c * 8 + half * 4
                        nz = sum(1 for g in range(g0, g0 + 4)
                                 if cfg.rg[g] > 0)
                        if nz == 0:
                            continue
                        ps = pss.tile([N_BIL, 512], F32, tag="ps")
                        for g in range(g0, g0 + 4):
                            R = cfg.rg[g]
                            if R == 0:
                                continue
                            sl = (g - g0) * 128
                            for p in range(R):
                                nc.tensor.matmul(
                                    ps[:, sl:sl + 128], wa_sb[:],
                                    at_block(blk),
                                    start=(p == 0), stop=(p == R - 1))
                                blk += 1
                        nc.scalar.activation(
                            st[:, half * 512:half * 512 + nz * 128],
                            ps[:, :nz * 128], AF.Copy)
                    nc.scalar.dma_start(out=sTd[c].ap(), in_=st[:])

            # ============ phase B: edge transform + S apply ================
            wn_sb = cp.tile([D, D], BF16)
            nc.sync.dma_start(out=wn_sb[:], in_=wnbr[:])
            b_sb = cp.tile([D, 1], F32)
            nc.sync.dma_start(out=b_sb[:], in_=bnbr[:])
            we_sb = cp.tile([N_RBF, D], BF16)
            nc.sync.dma_start(out=we_sb[:], in_=wes[:])
            t2_sb = cp.tile([D, N_BIL * D], BF16)
            nc.sync.dma_start(out=t2_sb[:], in_=t2[:])

            with tc.tile_pool(name="pb", bufs=2) as pb, \
                 tc.tile_pool(name="sbp", bufs=3) as sbp, \
                 tc.tile_pool(name="mjp", bufs=2) as mjp, \
                 tc.tile_pool(name="zp", bufs=2) as zp, \
                 tc.tile_pool(name="ofp", bufs=2) as ofp, \
                 tc.tile_pool(name="psmm", bufs=2, space="PSUM") as pmm, \
                 tc.tile_pool(name="psy", bufs=1, space="PSUM") as py:
                mj2 = of2 = None
                for c in range(NC):
                    s_bc = sbp.tile([128, N_BIL * 1024], BF16, tag="sbc")
                    nc.sync.dma_start(
                        out=s_bc[:],
                        in_=sTd[c].ap().unsqueeze(0).broadcast_to(
                            [128, N_BIL, 1024]))
                    er_sb = pb.tile([N_RBF, 1024], BF16, tag="er")
                    nc.scalar.dma_start(out=er_sb[:],
                                      in_=erbf[:, c * 1024:(c + 1) * 1024])
                    te_ps = pmm.tile([128, 1024], F32, tag="mm")
                    for n in range(2):
                        nc.tensor.matmul(
                            te_ps[:, n * 512:(n + 1) * 512],
                            we_sb[:], er_sb[:, n * 512:(n + 1) * 512],
                            start=True, stop=True)
                    te_sb = pb.tile([128, 1024], BF16, tag="te")
                    nc.scalar.activation(te_sb[:], te_ps[:], AF.Copy)
                    if c % 2 == 0:
                        mj2 = mjp.tile([128, 2048], BF16, tag="mj")
                        cend = min(c + 2, NC)
                        nc.scalar.dma_start(
                            out=mj2[:, :(cend - c) * 1024],
                            in_=mjiT[:, c * 1024:cend * 1024])
                    mj = mj2[:, (c % 2) * 1024:(c % 2 + 1) * 1024]
                    tm_ps = pmm.tile([128, 1024], F32, tag="mm")
                    for n in range(2):
                        nc.tensor.matmul(
                            tm_ps[:, n * 512:(n + 1) * 512],
                            wn_sb[:], mj[:, n * 512:(n + 1) * 512],
                            start=True, stop=True)
                    tm_sb = pb.tile([128, 1024], BF16, tag="tm")
                    nc.scalar.activation(tm_sb[:], tm_ps[:], AF.Silu,
                                         bias=b_sb[:, 0:1])
                    me_sb = pb.tile([128, 1024], BF16, tag="me")
                    nc.vector.tensor_mul(me_sb[:], tm_sb[:], te_sb[:])

                    z = zp.tile([128, N_BIL * 1024], BF16, tag="z")
                    for j in range(N_BIL):
                        nc.vector.tensor_mul(
                            z[:, j * 1024:(j + 1) * 1024], me_sb[:],
                            s_bc[:, j * 1024:(j + 1) * 1024])
                    ot = py.tile([128, 1024], F32, tag="ot")
                    for h in range(2):
                        for j in range(N_BIL):
                            nc.tensor.matmul(
                                ot[:, h * 512:(h + 1) * 512],
                                t2_sb[:, j * 128:(j + 1) * 128],
                                z[:, j * 1024 + h * 512:j * 1024 + (h + 1) * 512],
                                start=(j == 0), stop=(j == N_BIL - 1))
                    if c % 2 == 0:
                        of2 = ofp.tile([128, 2048], BF16, tag="of")
                    nc.scalar.activation(
                        of2[:, (c % 2) * 1024:(c % 2 + 1) * 1024], ot[:],
                        AF.Copy)
                    if c % 2 == 1 or c == NC - 1:
                        c0 = c - (c % 2)
                        nc.scalar.dma_start(
                            out=outd[:, c0 * 1024:(c + 1) * 1024],
                            in_=of2[:, :(c + 1 - c0) * 1024])
    nc.finalize()
    return nc


# ----------------------------------------------------------------------------
# host-side sharding / unsharding
# ----------------------------------------------------------------------------

def make_cfg(kj, n_edges, ev=25_000, ep=26_624):
    n_cores = (n_edges + ev - 1) // ev
    owner = np.minimum(kj // ev, n_cores - 1)
    ng = ep // 128
    rg = np.zeros(ng, np.int64)
    for c in range(n_cores):
        loc = kj[owner == c] - c * ev
        cnt = np.bincount(loc, minlength=ev)
        s = np.zeros(ep, np.int64)
        s[:ev] = np.sort(cnt)[::-1]
        gmax = s.reshape(ng, 128).max(axis=1)
        rg = np.maximum(rg, (gmax + 1) // 2)
    return Cfg(ev, ep, tuple(int(r) for r in rg))


def prep_in_maps(cfg: Cfg, m_ji, nbr_list, angle_list, e_rbf, a_sbf, kj_idx,
                 W_nbr, b_nbr, W_e, W_a, final_w):
    del nbr_list, angle_list
    m_ji = np.asarray(m_ji, np.float32)
    e_rbf = np.asarray(e_rbf, np.float32)
    a_sbf = np.asarray(a_sbf, np.float32)
    kj = np.asarray(kj_idx).astype(np.int64)
    W_nbr = np.asarray(W_nbr, np.float32)
    b_nbr = np.asarray(b_nbr, np.float32)
    W_e = np.asarray(W_e, np.float32)
    W_a = np.asarray(W_a, np.float32)
    final_w = np.asarray(final_w, np.float32)

    n_edges = m_ji.shape[0]
    ev = cfg.e_valid
    ep = cfg.e_pad
    n_cores = (n_edges + ev - 1) // ev
    owner = np.minimum(kj // ev, n_cores - 1)

    wa2 = np.zeros((AT_P, N_BIL), np.float32)
    wa2[0:A_DIM] = W_a
    wa2[A_DIM:2 * A_DIM] = W_a
    t2 = np.ascontiguousarray(final_w.transpose(2, 1, 0).reshape(D, N_BIL * D))
    bn = np.ascontiguousarray(b_nbr.reshape(D, 1))

    in_maps = []
    perms = []
    for c in range(n_cores):
        sel = np.nonzero(owner == c)[0]
        loc = kj[sel] - c * ev
        cnt = np.bincount(loc, minlength=ev)
        edge_order = np.argsort(-cnt, kind="stable")     # slot -> local edge
        slot_of_edge = np.empty(ev, np.int64)
        slot_of_edge[edge_order] = np.arange(ev)
        ang_slot = slot_of_edge[loc]
        order = np.argsort(ang_slot, kind="stable")
        rows = sel[order]                 # a_sbf row per (slot-sorted) token
        cnt_slot = np.bincount(ang_slot, minlength=ep)
        starts = np.concatenate([[0], np.cumsum(cnt_slot)])

        at = np.zeros((AT_P, cfg.at_cols), np.float32)
        col = 0
        for g in range(cfg.n_groups):
            sl = np.arange(g * 128, (g + 1) * 128)
            csl = cnt_slot[sl]
            for p in range(cfg.rg[g]):
                for half, r in ((0, 2 * p), (1, 2 * p + 1)):
                    has = np.nonzero(csl > r)[0]
                    if len(has):
                        tok = starts[sl[has]] + r
                        at[half * A_DIM:(half + 1) * A_DIM,
                           col + has] = a_sbf[rows[tok]].T
                col += 128
        assert col == cfg.n_blocks * 128

        e0, e1 = c * ev, min((c + 1) * ev, n_edges)
        mjiT = np.zeros((D, ep), np.float32)
        mjiT[:, :e1 - e0] = m_ji[e0:e1][edge_order[:e1 - e0]].T
        erbfT = np.zeros((N_RBF, ep), np.float32)
        erbfT[:, :e1 - e0] = e_rbf[e0:e1][edge_order[:e1 - e0]].T

        bf = mybir.dt.np(BF16)
        im = {
            "a_t": at.astype(bf), "mji_t": mjiT.astype(bf),
            "erbf_t": erbfT.astype(bf), "w_nbr": W_nbr.astype(bf),
            "b_nbr": bn, "w_e": W_e.astype(bf), "w_a2": wa2.astype(bf),
            "t2": t2.astype(bf),
        }
        in_maps.append(im)
        perms.append(edge_order)
    return in_maps, perms


def gather_output(cfg: Cfg, results, perms, n_edges):
    ev = cfg.e_valid
    out = np.empty((n_edges, D), np.float32)
    for c, r in enumerate(results):
        e0, e1 = c * ev, min((c + 1) * ev, n_edges)
        dev = np.asarray(r["out"]).astype(np.float32)       # [D, EP]
        out[e0 + perms[c][:e1 - e0]] = dev[:, :e1 - e0].T
    return out


_NC_CACHE = {}


def run_on_hw(inputs, cfg=None, trace=False, trace_cores=None):
    kj = np.asarray(inputs["kj_idx"]).astype(np.int64)
    if cfg is None:
        cfg = make_cfg(kj, inputs["m_ji"].shape[0])
    key = cfg.key()
    if key not in _NC_CACHE:
        _NC_CACHE[key] = build_nc(cfg)
    nc = _NC_CACHE[key]
    in_maps, perms = prep_in_maps(cfg, **inputs)
    res = bass_utils.run_bass_kernel_spmd(
        nc, in_maps, core_ids=list(range(len(in_maps))),
        trace=trace, trace_cores=trace_cores)
    out = gather_output(cfg, res.results, perms, inputs["m_ji"].shape[0])
    return out, res


def kernel(**inputs) -> np.ndarray:
    out, _ = run_on_hw(inputs)
    return out


# revision 13
# speedup vs baseline: 8.9471x; 1.0813x over previous
"""Trainium2 Bass kernel for the DimeNet-style directed-message block.

Reference computation (W = n_angles, E = n_edges, D = 128, A = 49, J = 8):
    m_kj     = m_ji[kj_idx]                          # [W, D]
    transf_m = silu(m_kj @ W_nbr + b_nbr)            # [W, D]
    transf_e = e_rbf[kj_idx] @ W_e                   # [W, D]
    m_and_e  = transf_m * transf_e                   # [W, D]
    transf_a = a_sbf @ W_a                           # [W, J]
    out[w,i] = sum_{j,l} transf_a[w,j] m_and_e[w,l] final_w[i,j,l]
    final    = segment_sum(out, kj_idx, E)           # [E, D]

Algebraic refactor: the segment sum commutes through the bilinear form:
    me       = silu(m_ji @ W_nbr + b) * (e_rbf @ W_e)        # [E, D]
    S        = segment_sum(a_sbf @ W_a, kj_idx, E)           # [E, J]
    final[e] = sum_j S[e,j] * (me[e] @ final_w[:,j,:].T)     # [E, D]

S without scatter: edges are sharded contiguously (25000/core, angles
binned by owner core kj // 25000) and permuted within the core by
descending angle multiplicity.  Each 128-edge group g gets a static
rank-pair count rg[g] (cross-core max); the host packs the angles as
[98, 128] blocks (rank 2p in partitions 0:49, 2p+1 in 49:98), so

    S^T[:, group g] = sum_p [W_a; W_a]^T @ aT_block(g, p)    # [8, 128]

is a plain PSUM accumulation (feature-major S).  Descending sort makes the
rank profile a staircase: ~12% padding, no overflow level, no scatter.

The apply keeps everything feature-major.  S^T round-trips through DRAM and
is re-read with a partition-broadcast DMA (each SBUF partition reads the
same DRAM bytes), giving s_bc[l, (j,e)] = S[e,j] on all 128 partitions.
Then per chunk of 1024 edges:
    z_j  = me * s_bc_j                  # DVE bf16 2x, feature-major
    outT = sum_j final_w[:,j,:] @ z_j   # PSUM accumulation over j
and outT [D, E] is written bf16; the host transposes/casts/unpermutes.
"""

import numpy as np

import concourse.bass as bass
import concourse.mybir as mybir
import concourse.tile as tile
from concourse import bacc, bass_utils

F32 = mybir.dt.float32
BF16 = mybir.dt.bfloat16
AF = mybir.ActivationFunctionType
OP = mybir.AluOpType

D = 128
A_DIM = 49
N_RBF = 6
N_BIL = 8
N_CORES = 8
AT_P = 2 * A_DIM          # 98 partitions: even rank 0:49, odd rank 49:98
AT_TILE = 4096            # aT stream tile width (cols); 32 blocks per tile


class Cfg:
    def __init__(self, e_valid, e_pad, rg):
        self.e_valid = e_valid
        self.e_pad = e_pad
        self.rg = tuple(int(r) for r in rg)      # rank-pairs per 128-edge group
        assert e_pad % 1024 == 0
        self.n_groups = e_pad // 128
        assert len(self.rg) == self.n_groups
        self.n_blocks = sum(self.rg)
        self.at_cols = ((self.n_blocks * 128 + AT_TILE - 1) // AT_TILE) * AT_TILE
        self.n_chunks = e_pad // 1024

    def key(self):
        return (self.e_valid, self.e_pad, self.rg)


def build_nc(cfg: Cfg):
    nc = bacc.Bacc(None)
    EP = cfg.e_pad
    NG = cfg.n_groups
    NC = cfg.n_chunks

    aT = nc.dram_tensor("a_t", [AT_P, cfg.at_cols], BF16, kind="ExternalInput")
    mjiT = nc.dram_tensor("mji_t", [D, EP], BF16, kind="ExternalInput")
    erbf = nc.dram_tensor("erbf_t", [N_RBF, EP], BF16, kind="ExternalInput")
    wnbr = nc.dram_tensor("w_nbr", [D, D], BF16, kind="ExternalInput")
    bnbr = nc.dram_tensor("b_nbr", [D, 1], F32, kind="ExternalInput")
    wes = nc.dram_tensor("w_e", [N_RBF, D], BF16, kind="ExternalInput")
    wa2 = nc.dram_tensor("w_a2", [AT_P, N_BIL], BF16, kind="ExternalInput")
    t2 = nc.dram_tensor("t2", [D, N_BIL * D], BF16, kind="ExternalInput")
    outd = nc.dram_tensor("out", [D, EP], BF16, kind="ExternalOutput")
    # chunk-major S^T spill, one tensor per chunk so phase B pipelines with A
    sTd = [nc.dram_tensor(f"s_t{c}", [N_BIL, 1024], BF16) for c in range(NC)]

    with tile.TileContext(nc) as tc:
        with tc.tile_pool(name="const", bufs=1) as cp:
            wa_sb = cp.tile([AT_P, N_BIL], BF16)
            nc.sync.dma_start(out=wa_sb[:], in_=wa2[:])
            wn_sb = cp.tile([D, D], BF16)
            nc.sync.dma_start(out=wn_sb[:], in_=wnbr[:])
            b_sb = cp.tile([D, 1], F32)
            nc.sync.dma_start(out=b_sb[:], in_=bnbr[:])
            we_sb = cp.tile([N_RBF, D], BF16)
            nc.sync.dma_start(out=we_sb[:], in_=wes[:])
            t2_sb = cp.tile([D, N_BIL * D], BF16)
            nc.sync.dma_start(out=t2_sb[:], in_=t2[:])

            with tc.tile_pool(name="pa", bufs=4) as pa, \
                 tc.tile_pool(name="stp", bufs=3) as stp, \
                 tc.tile_pool(name="pss", bufs=2, space="PSUM") as pss, \
                 tc.tile_pool(name="pb", bufs=2) as pb, \
                 tc.tile_pool(name="sbp", bufs=3) as sbp, \
                 tc.tile_pool(name="mjp", bufs=2) as mjp, \
                 tc.tile_pool(name="zp", bufs=2) as zp, \
                 tc.tile_pool(name="ofp", bufs=2) as ofp, \
                 tc.tile_pool(name="psmm", bufs=2, space="PSUM") as pmm, \
                 tc.tile_pool(name="psy", bufs=1, space="PSUM") as py:
                at_tiles = {}

                def at_block(b):
                    tk = b // (AT_TILE // 128)
                    if tk not in at_tiles:
                        t = pa.tile([AT_P, AT_TILE], BF16, tag="at")
                        nc.sync.dma_start(
                            out=t[:], in_=aT[:, tk * AT_TILE:(tk + 1) * AT_TILE])
                        at_tiles.clear()
                        at_tiles[tk] = t
                    off = (b % (AT_TILE // 128)) * 128
                    return at_tiles[tk][:, off:off + 128]

                blk_ctr = [0]

                def phase_a_chunk(c):
                    st = stp.tile([N_BIL, 1024], BF16, tag="st")
                    nzc = sum(1 for g in range(c * 8, c * 8 + 8)
                              if cfg.rg[g] > 0)
                    if nzc < 8:
                        nc.vector.memset(st[:], 0.0)
                    for half in range(2):
                        g0 = c * 8 + half * 4
                        nz = sum(1 for g in range(g0, g0 + 4)
                                 if cfg.rg[g] > 0)
                        if nz == 0:
                            continue
                        ps = pss.tile([N_BIL, 512], F32, tag="ps")
                        for g in range(g0, g0 + 4):
                            R = cfg.rg[g]
                            if R == 0:
                                continue
                            sl = (g - g0) * 128
                            for p in range(R):
                                nc.tensor.matmul(
                                    ps[:, sl:sl + 128], wa_sb[:],
                                    at_block(blk_ctr[0]),
                                    start=(p == 0), stop=(p == R - 1))
                                blk_ctr[0] += 1
                        nc.scalar.activation(
                            st[:, half * 512:half * 512 + nz * 128],
                            ps[:, :nz * 128], AF.Copy)
                    nc.scalar.dma_start(out=sTd[c].ap(), in_=st[:])

                state = {}

                def phase_b_chunk(c):
                    s_bc = sbp.tile([128, N_BIL * 1024], BF16, tag="sbc")
                    nc.sync.dma_start(
                        out=s_bc[:],
                        in_=sTd[c].ap().unsqueeze(0).broadcast_to(
                            [128, N_BIL, 1024]))
                    er_sb = pb.tile([N_RBF, 1024], BF16, tag="er")
                    nc.scalar.dma_start(out=er_sb[:],
                                        in_=erbf[:, c * 1024:(c + 1) * 1024])
                    te_ps = pmm.tile([128, 1024], F32, tag="mm")
                    for n in range(2):
                        nc.tensor.matmul(
                            te_ps[:, n * 512:(n + 1) * 512],
                            we_sb[:], er_sb[:, n * 512:(n + 1) * 512],
                            start=True, stop=True)
                    te_sb = pb.tile([128, 1024], BF16, tag="te")
                    nc.scalar.activation(te_sb[:], te_ps[:], AF.Copy)
                    if c % 2 == 0:
                        state["mj2"] = mjp.tile([128, 2048], BF16, tag="mj", name="mj2")
                        cend = min(c + 2, NC)
                        nc.scalar.dma_start(
                            out=state["mj2"][:, :(cend - c) * 1024],
                            in_=mjiT[:, c * 1024:cend * 1024])
                    mj = state["mj2"][:, (c % 2) * 1024:(c % 2 + 1) * 1024]
                    tm_ps = pmm.tile([128, 1024], F32, tag="mm")
                    for n in range(2):
                        nc.tensor.matmul(
                            tm_ps[:, n * 512:(n + 1) * 512],
                            wn_sb[:], mj[:, n * 512:(n + 1) * 512],
                            start=True, stop=True)
                    tm_sb = pb.tile([128, 1024], BF16, tag="tm")
                    nc.scalar.activation(tm_sb[:], tm_ps[:], AF.Silu,
                                         bias=b_sb[:, 0:1])
                    me_sb = pb.tile([128, 1024], BF16, tag="me")
                    nc.vector.tensor_mul(me_sb[:], tm_sb[:], te_sb[:])

                    z = zp.tile([128, N_BIL * 1024], BF16, tag="z")
                    for j in range(N_BIL):
                        nc.vector.tensor_mul(
                            z[:, j * 1024:(j + 1) * 1024], me_sb[:],
                            s_bc[:, j * 1024:(j + 1) * 1024])
                    ot = py.tile([128, 1024], F32, tag="ot")
                    for h in range(2):
                        for j in range(N_BIL):
                            nc.tensor.matmul(
                                ot[:, h * 512:(h + 1) * 512],
                                t2_sb[:, j * 128:(j + 1) * 128],
                                z[:, j * 1024 + h * 512:j * 1024 + (h + 1) * 512],
                                start=(j == 0), stop=(j == N_BIL - 1))
                    if c % 2 == 0:
                        state["of2"] = ofp.tile([128, 2048], BF16, tag="of", name="of2")
                    nc.scalar.activation(
                        state["of2"][:, (c % 2) * 1024:(c % 2 + 1) * 1024],
                        ot[:], AF.Copy)
                    if c % 2 == 1 or c == NC - 1:
                        c0 = c - (c % 2)
                        nc.scalar.dma_start(
                            out=outd[:, c0 * 1024:(c + 1) * 1024],
                            in_=state["of2"][:, :(c + 1 - c0) * 1024])

                LAG = 2
                for c in range(NC + LAG):
                    if c < NC:
                        phase_a_chunk(c)
                    if c >= LAG:
                        phase_b_chunk(c - LAG)
    nc.finalize()
    return nc


# ----------------------------------------------------------------------------
# host-side sharding / unsharding
# ----------------------------------------------------------------------------

def make_cfg(kj, n_edges, ev=25_000, ep=26_624):
    n_cores = (n_edges + ev - 1) // ev
    owner = np.minimum(kj // ev, n_cores - 1)
    ng = ep // 128
    rg = np.zeros(ng, np.int64)
    for c in range(n_cores):
        loc = kj[owner == c] - c * ev
        cnt = np.bincount(loc, minlength=ev)
        s = np.zeros(ep, np.int64)
        s[:ev] = np.sort(cnt)[::-1]
        gmax = s.reshape(ng, 128).max(axis=1)
        rg = np.maximum(rg, (gmax + 1) // 2)
    return Cfg(ev, ep, tuple(int(r) for r in rg))


def prep_in_maps(cfg: Cfg, m_ji, nbr_list, angle_list, e_rbf, a_sbf, kj_idx,
                 W_nbr, b_nbr, W_e, W_a, final_w):
    del nbr_list, angle_list
    m_ji = np.asarray(m_ji, np.float32)
    e_rbf = np.asarray(e_rbf, np.float32)
    a_sbf = np.asarray(a_sbf, np.float32)
    kj = np.asarray(kj_idx).astype(np.int64)
    W_nbr = np.asarray(W_nbr, np.float32)
    b_nbr = np.asarray(b_nbr, np.float32)
    W_e = np.asarray(W_e, np.float32)
    W_a = np.asarray(W_a, np.float32)
    final_w = np.asarray(final_w, np.float32)

    n_edges = m_ji.shape[0]
    ev = cfg.e_valid
    ep = cfg.e_pad
    n_cores = (n_edges + ev - 1) // ev
    owner = np.minimum(kj // ev, n_cores - 1)

    wa2 = np.zeros((AT_P, N_BIL), np.float32)
    wa2[0:A_DIM] = W_a
    wa2[A_DIM:2 * A_DIM] = W_a
    t2 = np.ascontiguousarray(final_w.transpose(2, 1, 0).reshape(D, N_BIL * D))
    bn = np.ascontiguousarray(b_nbr.reshape(D, 1))

    in_maps = []
    perms = []
    for c in range(n_cores):
        sel = np.nonzero(owner == c)[0]
        loc = kj[sel] - c * ev
        cnt = np.bincount(loc, minlength=ev)
        edge_order = np.argsort(-cnt, kind="stable")     # slot -> local edge
        slot_of_edge = np.empty(ev, np.int64)
        slot_of_edge[edge_order] = np.arange(ev)
        ang_slot = slot_of_edge[loc]
        order = np.argsort(ang_slot, kind="stable")
        rows = sel[order]                 # a_sbf row per (slot-sorted) token
        cnt_slot = np.bincount(ang_slot, minlength=ep)
        starts = np.concatenate([[0], np.cumsum(cnt_slot)])

        at = np.zeros((AT_P, cfg.at_cols), np.float32)
        col = 0
        for g in range(cfg.n_groups):
            sl = np.arange(g * 128, (g + 1) * 128)
            csl = cnt_slot[sl]
            for p in range(cfg.rg[g]):
                for half, r in ((0, 2 * p), (1, 2 * p + 1)):
                    has = np.nonzero(csl > r)[0]
                    if len(has):
                        tok = starts[sl[has]] + r
                        at[half * A_DIM:(half + 1) * A_DIM,
                           col + has] = a_sbf[rows[tok]].T
                col += 128
        assert col == cfg.n_blocks * 128

        e0, e1 = c * ev, min((c + 1) * ev, n_edges)
        mjiT = np.zeros((D, ep), np.float32)
        mjiT[:, :e1 - e0] = m_ji[e0:e1][edge_order[:e1 - e0]].T
        erbfT = np.zeros((N_RBF, ep), np.float32)
        erbfT[:, :e1 - e0] = e_rbf[e0:e1][edge_order[:e1 - e0]].T

        bf = mybir.dt.np(BF16)
        im = {
            "a_t": at.astype(bf), "mji_t": mjiT.astype(bf),
            "erbf_t": erbfT.astype(bf), "w_nbr": W_nbr.astype(bf),
            "b_nbr": bn, "w_e": W_e.astype(bf), "w_a2": wa2.astype(bf),
            "t2": t2.astype(bf),
        }
        in_maps.append(im)
        perms.append(edge_order)
    return in_maps, perms


def gather_output(cfg: Cfg, results, perms, n_edges):
    ev = cfg.e_valid
    out = np.empty((n_edges, D), np.float32)
    for c, r in enumerate(results):
        e0, e1 = c * ev, min((c + 1) * ev, n_edges)
        dev = np.asarray(r["out"]).astype(np.float32)       # [D, EP]
        out[e0 + perms[c][:e1 - e0]] = dev[:, :e1 - e0].T
    return out


_NC_CACHE = {}


def run_on_hw(inputs, cfg=None, trace=False, trace_cores=None):
    kj = np.asarray(inputs["kj_idx"]).astype(np.int64)
    if cfg is None:
        cfg = make_cfg(kj, inputs["m_ji"].shape[0])
    key = cfg.key()
    if key not in _NC_CACHE:
        _NC_CACHE[key] = build_nc(cfg)
    nc = _NC_CACHE[key]
    in_maps, perms = prep_in_maps(cfg, **inputs)
    res = bass_utils.run_bass_kernel_spmd(
        nc, in_maps, core_ids=list(range(len(in_maps))),
        trace=trace, trace_cores=trace_cores)
    out = gather_output(cfg, res.results, perms, inputs["m_ji"].shape[0])
    return out, res


def kernel(**inputs) -> np.ndarray:
    out, _ = run_on_hw(inputs)
    return out
